# revision 5
# baseline (speedup 1.0000x reference)
"""BEV-pool (lift-splat-shoot scatter-sum) Trainium2 Bass kernel, v3.

Strategy (DMA-byte-bound problem):
  Host (index math only): voxelize every frustum point (float32 mirror of
  the reference geometry), keep the ~16% in-bounds points, sort them by
  destination voxel, and split the sorted stream into 8 per-core ranges
  balanced by *chunk count*. A chunk is <=128 consecutive points spanning
  <=K distinct voxels. Features are staged to DRAM in bf16, partition-major
  ([128, nch*C]: chunk c, point p at [p, c*C:(c+1)*C]) so every DMA moves
  >=512B contiguous runs per partition (full modeled DMA rate).

  Device (per core, shared SPMD program): stream the point superblock in
  big piece DMAs (piece 0 carries the lid/iota metadata in its leading
  columns); build all chunk one-hots with a single DVE is_equal over
  broadcast APs; one tiny bf16 matmul per chunk scatter-sums the chunk
  into its private K-wide PSUM window (start=stop=True).

  Output is two-path. Chunks in the leading 64-chunk PSUM banks use
  column-oriented windows ([C, K] per chunk), per-piece Pool copies to
  bf16 stages, and one HWDGE DMA per bank -- all hidden under the input
  stream. The tail chunks (whose data arrives last) use row-oriented
  windows ([K, C] per chunk, slots on partitions) in per-16-chunk PSUM
  tiles; their outputs go through SWDGE dma_scatter_add descriptors that
  are PREPARED early and merely TRIGGERED when each tile's copy lands,
  skipping the ~1.4us HWDGE+DGE start latency on the post-stream tail.

  Host combine: np.add.at partial columns onto their voxels (a voxel's
  points may span chunks and cores), then scatter voxel rows into the
  dense BEV grid.
"""

import os
import numpy as np
import ml_dtypes

# ---- problem constants (from the reference nn.Module) ----
IMAGE_SIZE = (256, 704)
FEATURE_SIZE = (32, 88)
XBOUND = (-54.0, 54.0, 0.3)
YBOUND = (-54.0, 54.0, 0.3)
ZBOUND = (-10.0, 10.0, 20.0)
DBOUND = (1.0, 60.0, 1.0)

N_CORES = 8
P = 128          # matmul contraction dim = points per chunk
K = 8            # psum columns (distinct voxels) per chunk window
KT = 32          # tail one-hot width (4 chunks stacked in one window)
BANK_CHUNKS = 512 // K   # chunks per col-oriented PSUM bank
SC_GROUP = 4     # tail chunks per scatter group ([32, C] psum window)
SC_GROUPS = 8    # groups routed through the single prepared scatter
SC_RING = 3      # rotating row-oriented PSUM windows
PIECE_CHUNKS = 32        # target chunks per input DMA piece


def _host_geometry(img_trans, img_scale, lidar2img, B, N, D, H, W):
    """float32 numpy mirror of the reference get_geometry + voxelize."""
    dx = np.array([XBOUND[2], YBOUND[2], ZBOUND[2]], np.float32)
    bx = np.array([XBOUND[0] + XBOUND[2] / 2.0,
                   YBOUND[0] + YBOUND[2] / 2.0,
                   ZBOUND[0] + ZBOUND[2] / 2.0], np.float32)
    nx = [int((b[1] - b[0]) / b[2]) for b in (XBOUND, YBOUND, ZBOUND)]
    NX, NY, NZ = nx

    iH, iW = IMAGE_SIZE
    fH, fW = FEATURE_SIZE
    ds = np.arange(DBOUND[0], DBOUND[1], DBOUND[2], dtype=np.float32)
    xs = np.linspace(0.0, iW - 1, fW, dtype=np.float32)
    ys = np.linspace(0.0, iH - 1, fH, dtype=np.float32)
    assert ds.shape[0] == D and fH == H and fW == W

    fr = np.stack([
        np.broadcast_to(xs[None, None, :], (D, H, W)),
        np.broadcast_to(ys[None, :, None], (D, H, W)),
        np.broadcast_to(ds[:, None, None], (D, H, W)),
    ], axis=-1).astype(np.float32)                       # [D,H,W,3]

    pts = fr[None, None] + img_trans[:, :, None, None, None, :]
    d = pts[..., 2:3]
    xy = pts[..., :2] / img_scale[:, :, None, None, None, None]
    p4 = np.concatenate([xy * d, d, np.ones_like(d)], axis=-1)
    img2lidar = np.linalg.inv(lidar2img)
    geom = np.einsum('bnij,bndhwj->bndhwi', img2lidar, p4)[..., :3]
    geom = geom.astype(np.float32)
    vox = ((geom - (bx - dx / 2.0)) / dx).astype(np.int32)  # trunc toward 0
    mask = ((vox[..., 0] >= 0) & (vox[..., 0] < NX)
            & (vox[..., 1] >= 0) & (vox[..., 1] < NY)
            & (vox[..., 2] >= 0) & (vox[..., 2] < NZ))
    flat = (vox[..., 2] * NX + vox[..., 0]) * NY + vox[..., 1]
    flat = flat + np.arange(B, dtype=np.int32)[:, None, None, None, None] \
        * (NZ * NX * NY)
    flatm = np.where(mask, flat, -1).reshape(-1)
    return flatm, (NX, NY, NZ)


def _chunk_core(run_len, run_vox, T=None):
    """Greedy chunker over a run-list. Each chunk: <=P points, <=K voxels."""
    chunks = []
    cur = []
    cap, kv = P, 0
    i, off = 0, 0
    n = len(run_len)
    while i < n:
        left = run_len[i] - off
        if left == 0:
            i += 1
            off = 0
            continue
        if cap == 0 or (kv >= K):
            chunks.append(cur)
            cur, cap, kv = [], P, 0
            if T is not None and len(chunks) >= T:
                return chunks, i, off
        take = min(left, cap)
        cur.append((run_vox[i], take))
        cap -= take
        kv += 1
        off += take
        if off == run_len[i]:
            i += 1
            off = 0
        else:
            cap = 0
    if cur:
        chunks.append(cur)
    return chunks, i, 0


def _shard(run_len, run_vox):
    """Split the global run-list into N_CORES consecutive ranges minimizing
    the max per-core chunk count."""
    total = int(np.sum(run_len))
    lo, hi = -(-total // (P * N_CORES)), 2 * -(-total // P)

    def attempt(T):
        cores = []
        rl, rv = list(run_len), list(run_vox)
        for _ in range(N_CORES):
            if not rl:
                cores.append([])
                continue
            chunks, i, off = _chunk_core(rl, rv, T)
            cores.append(chunks)
            if off > 0:
                rl, rv = [rl[i] - off] + rl[i + 1:], rv[i:]
            else:
                rl, rv = rl[i:], rv[i:]
        return cores if not rl else None

    while lo < hi:
        mid = (lo + hi) // 2
        if attempt(mid) is not None:
            hi = mid
        else:
            lo = mid + 1
    cores = attempt(lo)
    assert cores is not None
    return cores, lo


def _layout(nch):
    """Chunk-space layout. Returns (nch_cols, banks, pieces):
    banks: col-oriented [lo,hi) spans (<=BANK_CHUNKS each) covering
    [0, nch_cols); chunks [nch_cols, nch) are the SC_GROUPS scatter groups
    of SC_GROUP chunks each; pieces: input DMA spans, never crossing a
    bank or group boundary, final piece = one group (tiny tail)."""
    n_sc = SC_GROUP * SC_GROUPS
    assert nch > n_sc or not SC_GROUPS
    nch_cols = nch - n_sc
    banks = []
    lo = 0
    while lo < nch_cols:
        banks.append((lo, min(lo + BANK_CHUNKS, nch_cols)))
        lo += BANK_CHUNKS

    pieces = []
    for lo, hi in banks:
        n = hi - lo
        nsub = max(1, round(n / PIECE_CHUNKS))
        for s in range(nsub):
            a, b = lo + (n * s) // nsub, lo + (n * (s + 1)) // nsub
            if b > a:
                pieces.append((a, b))
    if not SC_GROUPS:
        return nch_cols, banks, pieces
    # early scatter groups ride as the 2nd piece; the rest arrive last
    # so only their short trigger chain tails the stream
    early = 5 * SC_GROUP
    pieces.insert(1, (nch_cols, nch_cols + early))
    g = nch_cols + early
    n = nch - g
    if n > SC_GROUP:
        pieces.append((g, g + n - SC_GROUP))
        g += n - SC_GROUP
    pieces.append((g, nch))
    return nch_cols, banks, pieces


def _build_bass(nch, C, nch_cols, banks, pieces):
    import concourse.bass as bass
    import concourse.mybir as mybir
    import concourse.tile as tile
    from concourse import library_config

    f32 = mybir.dt.float32
    bf16 = mybir.dt.bfloat16
    i16 = mybir.dt.int16
    nrows = SC_GROUPS * SC_GROUP * K  # scatter tokens / dst rows
    tpp = -(-nrows // P)             # tokens per partition
    icols = -(-nrows // 16)          # idx columns
    W0 = nch + KT + icols            # metadata columns (lid | iota | idx)
    nc = bass.Bass()
    pts = nc.dram_tensor("pts", [P, W0 + nch * C], bf16, kind="ExternalInput")
    zz = nc.dram_tensor("zz", [max(1, nrows), P], bf16,
                        kind="ExternalInput")
    part = nc.dram_tensor("part", [C, nch_cols * K], bf16,
                          kind="ExternalOutput")
    part2 = nc.dram_tensor("part2", [max(1, nrows), P], bf16,
                           kind="ExternalOutput")
    dma_sem = nc.alloc_semaphore("sc_dma_sem")
    rdy_sem = nc.alloc_semaphore("sc_rdy")
    stage2 = nc.alloc_sbuf_tensor("stage2", [P, max(1, tpp) * C], bf16)

    loc = {}
    for b, (lo, hi) in enumerate(banks):
        for c in range(lo, hi):
            loc[c] = ("col", b)
    for c in range(nch_cols, nch):
        loc[c] = ("row", (c - nch_cols) // SC_GROUP)

    with tile.TileContext(nc) as tc:
        with tc.tile_pool(name="sb", bufs=1) as con, \
             tc.tile_pool(name="ps", bufs=1, space="PSUM") as ps:
            oh = con.tile([P, nch_cols * K], bf16, tag="oh")
            ntail = nch - nch_cols
            oh2 = (con.tile([P, ntail * KT], bf16, name="oh2",
                            tag="oh2")
                   if ntail else None)
            pcs = []
            for q, (a, b) in enumerate(pieces):
                w = (b - a) * C + (W0 if q == 0 else 0)
                t = con.tile([P, w], bf16, name=f"pc{q}", tag=f"pc{q}")
                pcs.append(t)
            accs = [ps.tile([P, (hi - lo) * K], f32, name=f"acc{b}",
                            tag=f"acc{b}")
                    for b, (lo, hi) in enumerate(banks)]
            stages = [con.tile([P, (hi - lo) * K], bf16, name=f"st{b}",
                               tag=f"st{b}")
                      for b, (lo, hi) in enumerate(banks)]
            ring = [ps.tile([P, C], f32, name=f"ring{r}", tag=f"ring{r}")
                    for r in range(SC_RING)]

            reloc = []
            if SC_GROUPS:
                nc.gpsimd.load_library(library_config.mlp)
                # zero the scatter-add destination (SWDGE path: keeps
                # the HWDGE free for the input pieces)
                zzi = nc.gpsimd.dma_start(out=part2[:, :], in_=zz[:])
                zz_name = zzi.ins.name

            for q, (a, b) in enumerate(pieces):
                if q == 0:
                    nc.sync.dma_start(out=pcs[q][:],
                                      in_=pts[:, 0:W0 + b * C])
                else:
                    nc.sync.dma_start(out=pcs[q][:],
                                      in_=pts[:, W0 + a * C:W0 + b * C])

            # scatter prep for the whole tail, emitted after the piece
            # DMAs (so its idx read is a RAW on piece 0, not a WAR); the
            # stage2 src read is deferred to the trigger
            if SC_GROUPS:
                st = stage2[:, :]
                in3 = bass.AP(st.tensor, st.offset,
                              [list(st.ap[0]), [C, tpp], [1, C]])
                idxs = pcs[0][:, nch + KT:nch + KT + icols].bitcast(i16)
                pr = nc.gpsimd.dma_scatter_add(
                    out_ap=part2[:, 0:C],
                    in_ap=in3,
                    idxs_ap=idxs,
                    num_idxs=nrows, num_idxs_reg=nrows, elem_size=C,
                    elem_step=P, prepare_only=True, sem=dma_sem)
                prep_name = pr.ins.name

            # one-hots in two DVE ops: oh[p, c*K+s] = (lid[p,c] == s)
            # for bank chunks; 32-wide stacked windows for tail chunks
            lid = pcs[0][:, 0:nch_cols]
            iota = pcs[0][:, nch:nch + KT]
            ap_lid = bass.AP(lid.tensor, lid.offset,
                             [list(lid.ap[0]), [1, nch_cols], [0, K]])
            ap_iota = bass.AP(iota.tensor, iota.offset,
                              [list(iota.ap[0]), [0, nch_cols], [1, K]])
            o = oh[:, :]
            ap_out = bass.AP(o.tensor, o.offset,
                             [list(o.ap[0]), [K, nch_cols], [1, K]])
            nc.vector.tensor_tensor(out=ap_out, in0=ap_lid, in1=ap_iota,
                                    op=mybir.AluOpType.is_equal)
            if ntail:
                lid2 = pcs[0][:, nch_cols:nch]
                ap_lid2 = bass.AP(lid2.tensor, lid2.offset,
                                  [list(lid2.ap[0]), [1, ntail], [0, KT]])
                ap_iota2 = bass.AP(iota.tensor, iota.offset,
                                   [list(iota.ap[0]), [0, ntail], [1, KT]])
                o2 = oh2[:, :]
                ap_out2 = bass.AP(o2.tensor, o2.offset,
                                  [list(o2.ap[0]), [KT, ntail], [1, KT]])
                nc.vector.tensor_tensor(out=ap_out2, in0=ap_lid2,
                                        in1=ap_iota2,
                                        op=mybir.AluOpType.is_equal)

            for q, (a, b) in enumerate(pieces):
                base = W0 if q == 0 else 0
                for c in range(a, b):
                    lc = c - a
                    lhs = pcs[q][:, base + lc * C:base + (lc + 1) * C]
                    kind, bi = loc[c]
                    if kind == "col":
                        blo = banks[bi][0]
                        off = (c - blo) * K
                        nc.tensor.matmul(out=accs[bi][:C, off:off + K],
                                         lhsT=lhs,
                                         rhs=oh[:, c * K:(c + 1) * K],
                                         start=True, stop=True)
                    else:
                        gc = (c - nch_cols) % SC_GROUP
                        tc_ = c - nch_cols
                        acc2 = ring[bi % SC_RING][:, :]
                        nc.tensor.matmul(
                            out=acc2[0:KT, :C],
                            lhsT=oh2[:, tc_ * KT:(tc_ + 1) * KT],
                            rhs=lhs,
                            start=(gc == 0), stop=(gc == SC_GROUP - 1))
                        if gc == SC_GROUP - 1:
                            # group complete: one copy into the token
                            # layout [128, tpp*C] (32-aligned base)
                            r0 = bi * KT
                            pr_, cb = r0 % P, r0 // P
                            od = bass.AP(stage2,
                                         pr_ * tpp * C + cb * C,
                                         [[tpp * C, KT], [1, C]])
                            if bi % 2 == 0:
                                ii = nc.scalar.activation(
                                    od, acc2[0:KT, :C],
                                    mybir.ActivationFunctionType.Copy)
                                ss = nc.scalar.sem_inc(rdy_sem, 1)
                            else:
                                ii = nc.vector.tensor_copy(
                                    out=od, in_=acc2[0:KT, :C])
                                ss = nc.vector.sem_inc(rdy_sem, 1)
                            reloc.append((ii.ins.name, ss.ins.name))
                            if bi == SC_GROUPS - 1:
                                tr = nc.gpsimd.trigger_dma(count=1)
                                trig_name = tr.ins.name
                kind, bi = loc[a]
                if kind == "col":
                    blo = banks[bi][0]
                    s0, s1 = (a - blo) * K, (b - blo) * K
                    if q % 2 == 0:
                        nc.vector.tensor_copy(out=stages[bi][:C, s0:s1],
                                              in_=accs[bi][:C, s0:s1])
                    else:
                        nc.scalar.activation(
                            stages[bi][:C, s0:s1], accs[bi][:C, s0:s1],
                            mybir.ActivationFunctionType.Copy)
                    if b == banks[bi][1]:
                        merged = (len(banks) >= 2 and bi == len(banks) - 1)
                        if merged:
                            # one DMA covers the last two banks' stages
                            for j in (len(banks) - 2, len(banks) - 1):
                                pass
                            b0 = len(banks) - 2
                            lo = banks[b0][0] * K
                            w0 = (banks[b0][1] - banks[b0][0]) * K
                            w1 = (banks[bi][1] - blo) * K
                            nc.scalar.dma_start(
                                out=part[:, lo:lo + w0],
                                in_=stages[b0][:C, :w0])
                            nc.scalar.dma_start(
                                out=part[:, blo * K:blo * K + w1],
                                in_=stages[bi][:C, :w1])
                        elif bi < len(banks) - 2:
                            lo = blo * K
                            w = (banks[bi][1] - blo) * K
                            nc.scalar.dma_start(out=part[:, lo:lo + w],
                                                in_=stages[bi][:C, :w])
            if SC_GROUPS:
                wg = nc.gpsimd.wait_ge(dma_sem, 16)
    if SC_GROUPS:
        _fix_swdge(nc, rdy_sem, dma_sem, reloc,
                   trig_name, prep_name, wg.ins.name, zz_name)
    return nc


def _fix_swdge(nc, rdy_sem, dma_sem, reloc,
               trig_name, prep_name, wg_name, zz_name):
    """Post-passes for the prepared-scatter machinery (name-based):
    0. relocate each rdy sem_inc EventSemaphore directly after its
       producing copy (the Tile scheduler hoists dep-less evsems);
    1. attach wait(rdy >= 3*SC_GROUPS) + wait(pre_sem) to the trigger
       (data deps hidden from Tile via the manual stage2 tensor);
    2. insert a pre_sem wait before the prep (idx table arrival) and
       move the wait_ge(dma_sem) right after the trigger;
    3. after each InstIncSwdgeSem (cost-model no-op), insert an explicit
       DMASW-lane bump so TimelineSim's epilogue drain isn't deadlocked
       (harmless double-bump for is_ge waits in exec mode)."""
    import concourse.mybir as mybir

    def mkwait(sem_num, name, val):
        return mybir.SyncWait(sync_type="semaphore", id=sem_num,
                              ant_name=name, wait_mode="sem-ge-imm",
                              wait_value=val, wait_reg=None)

    by_name = {}
    for bb in nc.m.functions[0].blocks:
        for inst in bb.instructions:
            by_name[inst.name] = inst

    trig = by_name[trig_name]
    ws = [mkwait(rdy_sem.num, "sc_rdy", SC_GROUPS)]
    if trig.sync_info is None:
        trig.sync_info = mybir.SyncInfo(on_wait=ws, on_update=[])
    else:
        trig.sync_info.on_wait = list(trig.sync_info.on_wait) + ws

    reloc_after = {prod: ev for prod, ev in reloc}
    ev_names = set(reloc_after.values())
    movable = {}
    for bb in nc.m.functions[0].blocks:
        for inst in bb.instructions:
            if inst.name in ev_names or inst.name == wg_name:
                movable[inst.name] = inst

    # relocate the prep cluster (IncSwdgeSem / reg moves / prep) to right
    # after the library load so its ~1us Pool engine time runs early, not
    # in the post-stream tail where the scheduler sank it
    for bb in nc.m.functions[0].blocks:
        insts = list(bb.instructions)
        try:
            pi = next(i for i, x in enumerate(insts)
                      if x.name == prep_name)
        except StopIteration:
            continue
        lo = pi
        while lo > 0 and type(insts[lo - 1]).__name__ in (
                "InstRegisterMove", "InstIncSwdgeSem"):
            lo -= 1
        cluster = insts[lo:pi + 1]
        prep = insts[pi]
        # keep Tile's engine-tick arithmetic intact: strip the tick from
        # the moved prep and fire it from a dummy at the old position
        si = prep.sync_info
        tick = [u for u in si.on_update if u.ant_name != "sc_dma_sem"]
        si.on_update = [u for u in si.on_update
                        if u.ant_name == "sc_dma_sem"]
        dummy = mybir.InstEventSemaphore(name="preptick", ins=[], outs=[])
        dummy.engine = prep.engine
        dummy.sync_info = mybir.SyncInfo(on_wait=[], on_update=tick)
        nc.inst_map[dummy.name] = dummy
        rest = insts[:lo] + [dummy] + insts[pi + 1:]
        try:
            li = next(i for i, x in enumerate(rest)
                      if x.name == zz_name)
            insts = rest[:li + 1] + cluster + rest[li + 1:]
        except StopIteration:
            insts = cluster + rest
        try:
            bb.instructions = insts
        except Exception:
            bb.instructions[:] = insts
        break

    for bb in nc.m.functions[0].blocks:
        insts = [i for i in bb.instructions if i.name not in movable]
        out = []
        for inst in insts:
            out.append(inst)
            ev = reloc_after.get(inst.name)
            if ev is not None and ev in movable:
                out.append(movable[ev])
            if inst.name == trig_name:
                out.append(movable[wg_name])
            # DMASW lane sems are a cost-model no-op at prep time (the
            # IncSwdgeSem bump never fires in TimelineSim) -- rewire any
            # wait on them to the descriptor-baked completion sem, which
            # fires in both sims
            si = inst.sync_info
            if si is not None and type(inst).__name__ == "InstDrain":
                for w in si.on_wait:
                    if w.ant_name and "DMASW" in w.ant_name:
                        w.id = dma_sem.num
                        w.ant_name = "sc_dma_sem"
                        w.wait_value = 16
        try:
            bb.instructions = out
        except Exception:
            bb.instructions[:] = out
    return nc


def _split_multi_waits(nc):
    """Walrus codegen allows a single sync-wait slot per instruction struct;
    hoist all but the last wait of any multi-wait instruction onto preceding
    single-wait EventSemaphores on the same engine queue."""
    import concourse.mybir as mybir

    k = 0
    for bb in nc.m.functions[0].blocks:
        new = []
        changed = False
        for inst in bb.instructions:
            si = inst.sync_info
            if si is not None and si.on_wait and len(si.on_wait) > 1:
                waits = list(si.on_wait)
                for w in waits[:-1]:
                    ev = mybir.InstEventSemaphore(
                        name=f"wsplit-{k}", ins=[], outs=[])
                    k += 1
                    ev.engine = inst.engine
                    ev.sync_info = mybir.SyncInfo(on_wait=[w], on_update=[])
                    nc.inst_map[ev.name] = ev
                    new.append(ev)
                si.on_wait = [waits[-1]]
                changed = True
            new.append(inst)
        if changed:
            try:
                bb.instructions = new
            except Exception:
                bb.instructions[:] = new
    return nc


def _prepare(feats, img_trans, img_scale, lidar2img):
    """Host-side indexing: geometry, sort, shard, materialize per-core
    arrays."""
    B, N, D, H, W, C = feats.shape
    npt = B * N * D * H * W

    flatm, (NX, NY, NZ) = _host_geometry(img_trans, img_scale, lidar2img,
                                         B, N, D, H, W)
    idx = np.nonzero(flatm >= 0)[0]
    keys = flatm[idx]
    order = np.argsort(keys, kind="stable")
    pidx = idx[order]
    vs = keys[order]
    uvox, run_start = np.unique(vs, return_index=True)
    run_len = np.diff(np.concatenate([run_start, [len(vs)]])).astype(int)
    run_vox = np.arange(len(uvox))

    cores, nch = _shard(list(run_len), list(run_vox))

    feats2 = feats.reshape(npt, C)
    sorted_feats = feats2[pidx].astype(ml_dtypes.bfloat16)

    iota_np = np.broadcast_to(
        np.arange(KT, dtype=np.float32)[None, :], (P, KT))
    nch_cols = nch - SC_GROUP * SC_GROUPS

    in_maps = []
    colmaps = []
    pos = 0
    for core in range(N_CORES):
        chunks = cores[core]
        arr = np.zeros((nch, P, C), ml_dtypes.bfloat16)
        lid = np.full((P, nch), -1.0, np.float32)
        colmap = np.full((nch, K), -1, np.int64)
        for c, segs in enumerate(chunks):
            n = sum(t for _, t in segs)
            arr[c, :n] = sorted_feats[pos:pos + n]
            # tail chunks stack SC_GROUP chunks into one [KT, C] psum
            # window: offset the local slot ids by gc*K
            soff = ((c - nch_cols) % SC_GROUP) * K if c >= nch_cols else 0
            o = 0
            for s, (v, t) in enumerate(segs):
                lid[o:o + t, c] = soff + s
                colmap[c, s] = v
                o += t
            pos += n
        ptsd = arr.transpose(1, 0, 2).reshape(P, nch * C)
        idx16 = np.ascontiguousarray(_aux_np()).view(ml_dtypes.bfloat16)
        meta = np.concatenate(
            [lid.astype(ml_dtypes.bfloat16),
             iota_np.astype(ml_dtypes.bfloat16), idx16], axis=1)
        pts_np = np.ascontiguousarray(np.concatenate([meta, ptsd], axis=1))
        in_maps.append({"pts": pts_np,
                        "zz": np.zeros((max(1, SC_GROUPS * SC_GROUP * K),
                                        P), ml_dtypes.bfloat16)})
        colmaps.append(colmap)
    assert pos == len(vs)
    return in_maps, colmaps, nch, uvox, (NX, NY, NZ), C, B


def _aux_np():
    """Scatter idx table: token i -> part2 row i, wrapped
    16-partition-minor; pad slots -1 (trailing, ignored), unused
    partitions 0."""
    nrows = SC_GROUPS * SC_GROUP * K
    icols = -(-nrows // 16)
    aux = np.zeros((P, icols), np.int16)
    for i in range(icols * 16):
        aux[i % 16, i // 16] = i if i < nrows else -1
    return aux


def _assemble(res_core, nch, nch_cols, C):
    """[C, nch*K] fp32 partial from the two output paths of one core."""
    full = np.zeros((C, nch * K), np.float32)
    full[:, :nch_cols * K] = np.asarray(
        res_core["part"], np.float32)[:, :nch_cols * K]
    nsc = SC_GROUPS * SC_GROUP * K
    if nsc:
        blk = np.asarray(res_core["part2"], np.float32)[:nsc, :C]
        full[:, nch_cols * K:nch_cols * K + nsc] = blk.T
    return full


def _combine(parts, colmaps, uvox, dims, C, B):
    NX, NY, NZ = dims
    nu = len(uvox)
    acc = np.zeros((nu + 1, C), np.float32)
    for part, colmap in zip(parts, colmaps):
        cm = colmap.reshape(-1).copy()
        cm[cm < 0] = nu
        np.add.at(acc, cm, np.asarray(part, np.float32).T)
    total = acc[:nu].T

    out = np.zeros((B, NZ * C, NX, NY), np.float32)
    gsz = NZ * NX * NY
    b_u = uvox // gsz
    r_u = uvox % gsz
    z_u = r_u // (NX * NY)
    xy_u = r_u % (NX * NY)
    ov = out.reshape(B, NZ, C, NX * NY)
    ov[b_u, z_u, :, xy_u] = total.T
    return out


def kernel(feats, img_trans, img_scale, lidar2img):
    from concourse import bass_utils

    feats = np.ascontiguousarray(feats, dtype=np.float32)
    img_trans = np.asarray(img_trans, dtype=np.float32)
    img_scale = np.asarray(img_scale, dtype=np.float32)
    lidar2img = np.asarray(lidar2img, dtype=np.float32)
    B, N, D, H, W, C = feats.shape

    in_maps, colmaps, nch, uvox, dims, C, B = _prepare(
        feats, img_trans, img_scale, lidar2img)
    if len(uvox) == 0:
        NX, NY, NZ = dims
        return np.zeros((B, NZ * C, NX, NY), np.float32)

    nch_cols, banks, pieces = _layout(nch)
    nc = _build_bass(nch, C, nch_cols, banks, pieces)
    _split_multi_waits(nc)

    if bool(int(os.environ.get("BEV_TIMELINE", "0"))):
        from concourse.timeline_sim import TimelineSim
        t_ns = TimelineSim(nc).simulate()
        print(f"HW exec time: {t_ns:.0f} ns")

    res = bass_utils.run_bass_kernel_spmd(
        nc, in_maps, core_ids=list(range(N_CORES)))
    parts = [_assemble(r, nch, nch_cols, C) for r in res.results]
    return _combine(parts, colmaps, uvox, dims, C, B)


# revision 6
# speedup vs baseline: 1.0108x; 1.0108x over previous
"""BEV-pool (lift-splat-shoot scatter-sum) Trainium2 Bass kernel, v3.

Strategy (DMA-byte-bound problem):
  Host (index math only): voxelize every frustum point (float32 mirror of
  the reference geometry), keep the ~16% in-bounds points, sort them by
  destination voxel, and split the sorted stream into 8 per-core ranges
  balanced by *chunk count*. A chunk is <=128 consecutive points spanning
  <=K distinct voxels. Features are staged to DRAM in bf16, partition-major
  ([128, nch*C]: chunk c, point p at [p, c*C:(c+1)*C]) so every DMA moves
  >=512B contiguous runs per partition (full modeled DMA rate).

  Device (per core, shared SPMD program): stream the point superblock in
  big piece DMAs (piece 0 carries the lid/iota metadata in its leading
  columns); build all chunk one-hots with a single DVE is_equal over
  broadcast APs; one tiny bf16 matmul per chunk scatter-sums the chunk
  into its private K-wide PSUM window (start=stop=True).

  Output is two-path. Chunks in the leading 64-chunk PSUM banks use
  column-oriented windows ([C, K] per chunk), per-piece Pool copies to
  bf16 stages, and one HWDGE DMA per bank -- all hidden under the input
  stream. The tail chunks (whose data arrives last) use row-oriented
  windows ([K, C] per chunk, slots on partitions) in per-16-chunk PSUM
  tiles; their outputs go through SWDGE dma_scatter_add descriptors that
  are PREPARED early and merely TRIGGERED when each tile's copy lands,
  skipping the ~1.4us HWDGE+DGE start latency on the post-stream tail.

  Host combine: np.add.at partial columns onto their voxels (a voxel's
  points may span chunks and cores), then scatter voxel rows into the
  dense BEV grid.
"""

import os
import numpy as np
import ml_dtypes

# ---- problem constants (from the reference nn.Module) ----
IMAGE_SIZE = (256, 704)
FEATURE_SIZE = (32, 88)
XBOUND = (-54.0, 54.0, 0.3)
YBOUND = (-54.0, 54.0, 0.3)
ZBOUND = (-10.0, 10.0, 20.0)
DBOUND = (1.0, 60.0, 1.0)

N_CORES = 8
P = 128          # matmul contraction dim = points per chunk
K = 8            # psum columns (distinct voxels) per chunk window
KT = 32          # tail one-hot width (4 chunks stacked in one window)
BANK_CHUNKS = 512 // K   # chunks per col-oriented PSUM bank
SC_GROUP = 4     # tail chunks per scatter group ([32, C] psum window)
SC_GROUPS = 0    # groups routed through the single prepared scatter
SC_RING = 3      # rotating row-oriented PSUM windows
PIECE_CHUNKS = 32        # target chunks per input DMA piece


def _host_geometry(img_trans, img_scale, lidar2img, B, N, D, H, W):
    """float32 numpy mirror of the reference get_geometry + voxelize."""
    dx = np.array([XBOUND[2], YBOUND[2], ZBOUND[2]], np.float32)
    bx = np.array([XBOUND[0] + XBOUND[2] / 2.0,
                   YBOUND[0] + YBOUND[2] / 2.0,
                   ZBOUND[0] + ZBOUND[2] / 2.0], np.float32)
    nx = [int((b[1] - b[0]) / b[2]) for b in (XBOUND, YBOUND, ZBOUND)]
    NX, NY, NZ = nx

    iH, iW = IMAGE_SIZE
    fH, fW = FEATURE_SIZE
    ds = np.arange(DBOUND[0], DBOUND[1], DBOUND[2], dtype=np.float32)
    xs = np.linspace(0.0, iW - 1, fW, dtype=np.float32)
    ys = np.linspace(0.0, iH - 1, fH, dtype=np.float32)
    assert ds.shape[0] == D and fH == H and fW == W

    fr = np.stack([
        np.broadcast_to(xs[None, None, :], (D, H, W)),
        np.broadcast_to(ys[None, :, None], (D, H, W)),
        np.broadcast_to(ds[:, None, None], (D, H, W)),
    ], axis=-1).astype(np.float32)                       # [D,H,W,3]

    pts = fr[None, None] + img_trans[:, :, None, None, None, :]
    d = pts[..., 2:3]
    xy = pts[..., :2] / img_scale[:, :, None, None, None, None]
    p4 = np.concatenate([xy * d, d, np.ones_like(d)], axis=-1)
    img2lidar = np.linalg.inv(lidar2img)
    geom = np.einsum('bnij,bndhwj->bndhwi', img2lidar, p4)[..., :3]
    geom = geom.astype(np.float32)
    vox = ((geom - (bx - dx / 2.0)) / dx).astype(np.int32)  # trunc toward 0
    mask = ((vox[..., 0] >= 0) & (vox[..., 0] < NX)
            & (vox[..., 1] >= 0) & (vox[..., 1] < NY)
            & (vox[..., 2] >= 0) & (vox[..., 2] < NZ))
    flat = (vox[..., 2] * NX + vox[..., 0]) * NY + vox[..., 1]
    flat = flat + np.arange(B, dtype=np.int32)[:, None, None, None, None] \
        * (NZ * NX * NY)
    flatm = np.where(mask, flat, -1).reshape(-1)
    return flatm, (NX, NY, NZ)


def _chunk_core(run_len, run_vox, T=None):
    """Greedy chunker over a run-list. Each chunk: <=P points, <=K voxels."""
    chunks = []
    cur = []
    cap, kv = P, 0
    i, off = 0, 0
    n = len(run_len)
    while i < n:
        left = run_len[i] - off
        if left == 0:
            i += 1
            off = 0
            continue
        if cap == 0 or (kv >= K):
            chunks.append(cur)
            cur, cap, kv = [], P, 0
            if T is not None and len(chunks) >= T:
                return chunks, i, off
        take = min(left, cap)
        cur.append((run_vox[i], take))
        cap -= take
        kv += 1
        off += take
        if off == run_len[i]:
            i += 1
            off = 0
        else:
            cap = 0
    if cur:
        chunks.append(cur)
    return chunks, i, 0


def _shard(run_len, run_vox):
    """Split the global run-list into N_CORES consecutive ranges minimizing
    the max per-core chunk count."""
    total = int(np.sum(run_len))
    lo, hi = -(-total // (P * N_CORES)), 2 * -(-total // P)

    def attempt(T):
        cores = []
        rl, rv = list(run_len), list(run_vox)
        for _ in range(N_CORES):
            if not rl:
                cores.append([])
                continue
            chunks, i, off = _chunk_core(rl, rv, T)
            cores.append(chunks)
            if off > 0:
                rl, rv = [rl[i] - off] + rl[i + 1:], rv[i:]
            else:
                rl, rv = rl[i:], rv[i:]
        return cores if not rl else None

    while lo < hi:
        mid = (lo + hi) // 2
        if attempt(mid) is not None:
            hi = mid
        else:
            lo = mid + 1
    cores = attempt(lo)
    assert cores is not None
    return cores, lo


def _layout(nch):
    """Chunk-space layout. Returns (nch_cols, banks, pieces):
    banks: col-oriented [lo,hi) spans (<=BANK_CHUNKS each) covering
    [0, nch_cols); chunks [nch_cols, nch) are the SC_GROUPS scatter groups
    of SC_GROUP chunks each; pieces: input DMA spans, never crossing a
    bank or group boundary, final piece = one group (tiny tail)."""
    n_sc = SC_GROUP * SC_GROUPS
    assert nch > n_sc or not SC_GROUPS
    nch_cols = nch - n_sc
    banks = []
    lo = 0
    while lo < nch_cols:
        banks.append((lo, min(lo + BANK_CHUNKS, nch_cols)))
        lo += BANK_CHUNKS

    pieces = []
    for lo, hi in banks:
        n = hi - lo
        nsub = max(1, round(n / PIECE_CHUNKS))
        for s in range(nsub):
            a, b = lo + (n * s) // nsub, lo + (n * (s + 1)) // nsub
            if b > a:
                pieces.append((a, b))
    if not SC_GROUPS:
        return nch_cols, banks, pieces
    # early scatter groups ride as the 2nd piece; the rest arrive last
    # so only their short trigger chain tails the stream
    early = 5 * SC_GROUP
    pieces.insert(1, (nch_cols, nch_cols + early))
    g = nch_cols + early
    n = nch - g
    if n > SC_GROUP:
        pieces.append((g, g + n - SC_GROUP))
        g += n - SC_GROUP
    pieces.append((g, nch))
    return nch_cols, banks, pieces


def _build_bass(nch, C, nch_cols, banks, pieces):
    import concourse.bass as bass
    import concourse.mybir as mybir
    import concourse.tile as tile
    from concourse import library_config

    f32 = mybir.dt.float32
    bf16 = mybir.dt.bfloat16
    i16 = mybir.dt.int16
    nrows = SC_GROUPS * SC_GROUP * K  # scatter tokens / dst rows
    tpp = -(-nrows // P)             # tokens per partition
    icols = -(-nrows // 16)          # idx columns
    W0 = nch + KT + icols            # metadata columns (lid | iota | idx)
    nc = bass.Bass()
    pts = nc.dram_tensor("pts", [P, W0 + nch * C], bf16, kind="ExternalInput")
    zz = nc.dram_tensor("zz", [max(1, nrows), P], bf16,
                        kind="ExternalInput")
    part = nc.dram_tensor("part", [C, nch_cols * K], bf16,
                          kind="ExternalOutput")
    part2 = nc.dram_tensor("part2", [max(1, nrows), P], bf16,
                           kind="ExternalOutput")
    dma_sem = nc.alloc_semaphore("sc_dma_sem")
    rdy_sem = nc.alloc_semaphore("sc_rdy")
    stage2 = nc.alloc_sbuf_tensor("stage2", [P, max(1, tpp) * C], bf16)

    loc = {}
    for b, (lo, hi) in enumerate(banks):
        for c in range(lo, hi):
            loc[c] = ("col", b)
    for c in range(nch_cols, nch):
        loc[c] = ("row", (c - nch_cols) // SC_GROUP)

    with tile.TileContext(nc) as tc:
        with tc.tile_pool(name="sb", bufs=1) as con, \
             tc.tile_pool(name="ps", bufs=1, space="PSUM") as ps:
            oh = con.tile([P, nch_cols * K], bf16, tag="oh")
            ntail = nch - nch_cols
            oh2 = (con.tile([P, ntail * KT], bf16, tag="oh2")
                   if ntail else None)
            pcs = []
            for q, (a, b) in enumerate(pieces):
                w = (b - a) * C + (W0 if q == 0 else 0)
                t = con.tile([P, w], bf16, name=f"pc{q}", tag=f"pc{q}")
                pcs.append(t)
            accs = [ps.tile([P, (hi - lo) * K], f32, name=f"acc{b}",
                            tag=f"acc{b}")
                    for b, (lo, hi) in enumerate(banks)]
            stages = [con.tile([P, (hi - lo) * K], bf16, name=f"st{b}",
                               tag=f"st{b}")
                      for b, (lo, hi) in enumerate(banks)]
            ring = [ps.tile([P, C], f32, name=f"ring{r}", tag=f"ring{r}")
                    for r in range(SC_RING)]

            reloc = []
            if SC_GROUPS:
                nc.gpsimd.load_library(library_config.mlp)
                # zero the scatter-add destination (SWDGE path: keeps
                # the HWDGE free for the input pieces)
                zzi = nc.gpsimd.dma_start(out=part2[:, :], in_=zz[:])
                zz_name = zzi.ins.name

            for q, (a, b) in enumerate(pieces):
                if q == 0:
                    nc.sync.dma_start(out=pcs[q][:],
                                      in_=pts[:, 0:W0 + b * C])
                else:
                    nc.sync.dma_start(out=pcs[q][:],
                                      in_=pts[:, W0 + a * C:W0 + b * C])

            # scatter prep for the whole tail, emitted after the piece
            # DMAs (so its idx read is a RAW on piece 0, not a WAR); the
            # stage2 src read is deferred to the trigger
            if SC_GROUPS:
                st = stage2[:, :]
                in3 = bass.AP(st.tensor, st.offset,
                              [list(st.ap[0]), [C, tpp], [1, C]])
                idxs = pcs[0][:, nch + KT:nch + KT + icols].bitcast(i16)
                pr = nc.gpsimd.dma_scatter_add(
                    out_ap=part2[:, 0:C],
                    in_ap=in3,
                    idxs_ap=idxs,
                    num_idxs=nrows, num_idxs_reg=nrows, elem_size=C,
                    elem_step=P, prepare_only=True, sem=dma_sem)
                prep_name = pr.ins.name

            # one-hots in two DVE ops: oh[p, c*K+s] = (lid[p,c] == s)
            # for bank chunks; 32-wide stacked windows for tail chunks
            lid = pcs[0][:, 0:nch_cols]
            iota = pcs[0][:, nch:nch + KT]
            ap_lid = bass.AP(lid.tensor, lid.offset,
                             [list(lid.ap[0]), [1, nch_cols], [0, K]])
            ap_iota = bass.AP(iota.tensor, iota.offset,
                              [list(iota.ap[0]), [0, nch_cols], [1, K]])
            o = oh[:, :]
            ap_out = bass.AP(o.tensor, o.offset,
                             [list(o.ap[0]), [K, nch_cols], [1, K]])
            nc.vector.tensor_tensor(out=ap_out, in0=ap_lid, in1=ap_iota,
                                    op=mybir.AluOpType.is_equal)
            if ntail:
                lid2 = pcs[0][:, nch_cols:nch]
                ap_lid2 = bass.AP(lid2.tensor, lid2.offset,
                                  [list(lid2.ap[0]), [1, ntail], [0, KT]])
                ap_iota2 = bass.AP(iota.tensor, iota.offset,
                                   [list(iota.ap[0]), [0, ntail], [1, KT]])
                o2 = oh2[:, :]
                ap_out2 = bass.AP(o2.tensor, o2.offset,
                                  [list(o2.ap[0]), [KT, ntail], [1, KT]])
                nc.vector.tensor_tensor(out=ap_out2, in0=ap_lid2,
                                        in1=ap_iota2,
                                        op=mybir.AluOpType.is_equal)

            for q, (a, b) in enumerate(pieces):
                base = W0 if q == 0 else 0
                for c in range(a, b):
                    lc = c - a
                    lhs = pcs[q][:, base + lc * C:base + (lc + 1) * C]
                    kind, bi = loc[c]
                    if kind == "col":
                        blo = banks[bi][0]
                        off = (c - blo) * K
                        nc.tensor.matmul(out=accs[bi][:C, off:off + K],
                                         lhsT=lhs,
                                         rhs=oh[:, c * K:(c + 1) * K],
                                         start=True, stop=True)
                    else:
                        gc = (c - nch_cols) % SC_GROUP
                        tc_ = c - nch_cols
                        acc2 = ring[bi % SC_RING][:, :]
                        nc.tensor.matmul(
                            out=acc2[0:KT, :C],
                            lhsT=oh2[:, tc_ * KT:(tc_ + 1) * KT],
                            rhs=lhs,
                            start=(gc == 0), stop=(gc == SC_GROUP - 1))
                        if gc == SC_GROUP - 1:
                            # group complete: one copy into the token
                            # layout [128, tpp*C] (32-aligned base)
                            r0 = bi * KT
                            pr_, cb = r0 % P, r0 // P
                            od = bass.AP(stage2,
                                         pr_ * tpp * C + cb * C,
                                         [[tpp * C, KT], [1, C]])
                            if bi % 2 == 0:
                                ii = nc.scalar.activation(
                                    od, acc2[0:KT, :C],
                                    mybir.ActivationFunctionType.Copy)
                                ss = nc.scalar.sem_inc(rdy_sem, 1)
                            else:
                                ii = nc.vector.tensor_copy(
                                    out=od, in_=acc2[0:KT, :C])
                                ss = nc.vector.sem_inc(rdy_sem, 1)
                            reloc.append((ii.ins.name, ss.ins.name))
                            if bi == SC_GROUPS - 1:
                                tr = nc.gpsimd.trigger_dma(count=1)
                                trig_name = tr.ins.name
                kind, bi = loc[a]
                if kind == "col":
                    blo = banks[bi][0]
                    s0, s1 = (a - blo) * K, (b - blo) * K
                    if q % 2 == 0:
                        nc.vector.tensor_copy(out=stages[bi][:C, s0:s1],
                                              in_=accs[bi][:C, s0:s1])
                    else:
                        nc.scalar.activation(
                            stages[bi][:C, s0:s1], accs[bi][:C, s0:s1],
                            mybir.ActivationFunctionType.Copy)
                    if b == banks[bi][1]:
                        merged = (len(banks) >= 2 and bi == len(banks) - 1)
                        if merged:
                            # one DMA covers the last two banks' stages
                            for j in (len(banks) - 2, len(banks) - 1):
                                pass
                            b0 = len(banks) - 2
                            lo = banks[b0][0] * K
                            w0 = (banks[b0][1] - banks[b0][0]) * K
                            w1 = (banks[bi][1] - blo) * K
                            nc.scalar.dma_start(
                                out=part[:, lo:lo + w0],
                                in_=stages[b0][:C, :w0])
                            nc.scalar.dma_start(
                                out=part[:, blo * K:blo * K + w1],
                                in_=stages[bi][:C, :w1])
                        elif bi < len(banks) - 2:
                            lo = blo * K
                            w = (banks[bi][1] - blo) * K
                            nc.scalar.dma_start(out=part[:, lo:lo + w],
                                                in_=stages[bi][:C, :w])
            if SC_GROUPS:
                wg = nc.gpsimd.wait_ge(dma_sem, 16)
    if SC_GROUPS:
        _fix_swdge(nc, rdy_sem, dma_sem, reloc,
                   trig_name, prep_name, wg.ins.name, zz_name)
    return nc


def _fix_swdge(nc, rdy_sem, dma_sem, reloc,
               trig_name, prep_name, wg_name, zz_name):
    """Post-passes for the prepared-scatter machinery (name-based):
    0. relocate each rdy sem_inc EventSemaphore directly after its
       producing copy (the Tile scheduler hoists dep-less evsems);
    1. attach wait(rdy >= 3*SC_GROUPS) + wait(pre_sem) to the trigger
       (data deps hidden from Tile via the manual stage2 tensor);
    2. insert a pre_sem wait before the prep (idx table arrival) and
       move the wait_ge(dma_sem) right after the trigger;
    3. after each InstIncSwdgeSem (cost-model no-op), insert an explicit
       DMASW-lane bump so TimelineSim's epilogue drain isn't deadlocked
       (harmless double-bump for is_ge waits in exec mode)."""
    import concourse.mybir as mybir

    def mkwait(sem_num, name, val):
        return mybir.SyncWait(sync_type="semaphore", id=sem_num,
                              ant_name=name, wait_mode="sem-ge-imm",
                              wait_value=val, wait_reg=None)

    by_name = {}
    for bb in nc.m.functions[0].blocks:
        for inst in bb.instructions:
            by_name[inst.name] = inst

    trig = by_name[trig_name]
    ws = [mkwait(rdy_sem.num, "sc_rdy", SC_GROUPS)]
    if trig.sync_info is None:
        trig.sync_info = mybir.SyncInfo(on_wait=ws, on_update=[])
    else:
        trig.sync_info.on_wait = list(trig.sync_info.on_wait) + ws

    reloc_after = {prod: ev for prod, ev in reloc}
    ev_names = set(reloc_after.values())
    movable = {}
    for bb in nc.m.functions[0].blocks:
        for inst in bb.instructions:
            if inst.name in ev_names or inst.name == wg_name:
                movable[inst.name] = inst

    # relocate the prep cluster (IncSwdgeSem / reg moves / prep) to right
    # after the library load so its ~1us Pool engine time runs early, not
    # in the post-stream tail where the scheduler sank it
    for bb in nc.m.functions[0].blocks:
        insts = list(bb.instructions)
        try:
            pi = next(i for i, x in enumerate(insts)
                      if x.name == prep_name)
        except StopIteration:
            continue
        lo = pi
        while lo > 0 and type(insts[lo - 1]).__name__ in (
                "InstRegisterMove", "InstIncSwdgeSem"):
            lo -= 1
        cluster = insts[lo:pi + 1]
        prep = insts[pi]
        # keep Tile's engine-tick arithmetic intact: strip the tick from
        # the moved prep and fire it from a dummy at the old position
        si = prep.sync_info
        tick = [u for u in si.on_update if u.ant_name != "sc_dma_sem"]
        si.on_update = [u for u in si.on_update
                        if u.ant_name == "sc_dma_sem"]
        dummy = mybir.InstEventSemaphore(name="preptick", ins=[], outs=[])
        dummy.engine = prep.engine
        dummy.sync_info = mybir.SyncInfo(on_wait=[], on_update=tick)
        nc.inst_map[dummy.name] = dummy
        rest = insts[:lo] + [dummy] + insts[pi + 1:]
        try:
            li = next(i for i, x in enumerate(rest)
                      if x.name == zz_name)
            insts = rest[:li + 1] + cluster + rest[li + 1:]
        except StopIteration:
            insts = cluster + rest
        try:
            bb.instructions = insts
        except Exception:
            bb.instructions[:] = insts
        break

    for bb in nc.m.functions[0].blocks:
        insts = [i for i in bb.instructions if i.name not in movable]
        out = []
        for inst in insts:
            out.append(inst)
            ev = reloc_after.get(inst.name)
            if ev is not None and ev in movable:
                out.append(movable[ev])
            if inst.name == trig_name:
                out.append(movable[wg_name])
            # DMASW lane sems are a cost-model no-op at prep time (the
            # IncSwdgeSem bump never fires in TimelineSim) -- rewire any
            # wait on them to the descriptor-baked completion sem, which
            # fires in both sims
            si = inst.sync_info
            if si is not None and type(inst).__name__ == "InstDrain":
                for w in si.on_wait:
                    if w.ant_name and "DMASW" in w.ant_name:
                        w.id = dma_sem.num
                        w.ant_name = "sc_dma_sem"
                        w.wait_value = 16
        try:
            bb.instructions = out
        except Exception:
            bb.instructions[:] = out
    return nc


def _split_multi_waits(nc):
    """Walrus codegen allows a single sync-wait slot per instruction struct;
    hoist all but the last wait of any multi-wait instruction onto preceding
    single-wait EventSemaphores on the same engine queue."""
    import concourse.mybir as mybir

    k = 0
    for bb in nc.m.functions[0].blocks:
        new = []
        changed = False
        for inst in bb.instructions:
            si = inst.sync_info
            if si is not None and si.on_wait and len(si.on_wait) > 1:
                waits = list(si.on_wait)
                for w in waits[:-1]:
                    ev = mybir.InstEventSemaphore(
                        name=f"wsplit-{k}", ins=[], outs=[])
                    k += 1
                    ev.engine = inst.engine
                    ev.sync_info = mybir.SyncInfo(on_wait=[w], on_update=[])
                    nc.inst_map[ev.name] = ev
                    new.append(ev)
                si.on_wait = [waits[-1]]
                changed = True
            new.append(inst)
        if changed:
            try:
                bb.instructions = new
            except Exception:
                bb.instructions[:] = new
    return nc


def _prepare(feats, img_trans, img_scale, lidar2img):
    """Host-side indexing: geometry, sort, shard, materialize per-core
    arrays."""
    B, N, D, H, W, C = feats.shape
    npt = B * N * D * H * W

    flatm, (NX, NY, NZ) = _host_geometry(img_trans, img_scale, lidar2img,
                                         B, N, D, H, W)
    idx = np.nonzero(flatm >= 0)[0]
    keys = flatm[idx]
    order = np.argsort(keys, kind="stable")
    pidx = idx[order]
    vs = keys[order]
    uvox, run_start = np.unique(vs, return_index=True)
    run_len = np.diff(np.concatenate([run_start, [len(vs)]])).astype(int)
    run_vox = np.arange(len(uvox))

    cores, nch = _shard(list(run_len), list(run_vox))

    feats2 = feats.reshape(npt, C)
    sorted_feats = feats2[pidx].astype(ml_dtypes.bfloat16)

    iota_np = np.broadcast_to(
        np.arange(KT, dtype=np.float32)[None, :], (P, KT))
    nch_cols = nch - SC_GROUP * SC_GROUPS

    in_maps = []
    colmaps = []
    pos = 0
    for core in range(N_CORES):
        chunks = cores[core]
        arr = np.zeros((nch, P, C), ml_dtypes.bfloat16)
        lid = np.full((P, nch), -1.0, np.float32)
        colmap = np.full((nch, K), -1, np.int64)
        for c, segs in enumerate(chunks):
            n = sum(t for _, t in segs)
            arr[c, :n] = sorted_feats[pos:pos + n]
            # tail chunks stack SC_GROUP chunks into one [KT, C] psum
            # window: offset the local slot ids by gc*K
            soff = ((c - nch_cols) % SC_GROUP) * K if c >= nch_cols else 0
            o = 0
            for s, (v, t) in enumerate(segs):
                lid[o:o + t, c] = soff + s
                colmap[c, s] = v
                o += t
            pos += n
        ptsd = arr.transpose(1, 0, 2).reshape(P, nch * C)
        idx16 = np.ascontiguousarray(_aux_np()).view(ml_dtypes.bfloat16)
        meta = np.concatenate(
            [lid.astype(ml_dtypes.bfloat16),
             iota_np.astype(ml_dtypes.bfloat16), idx16], axis=1)
        pts_np = np.ascontiguousarray(np.concatenate([meta, ptsd], axis=1))
        in_maps.append({"pts": pts_np,
                        "zz": np.zeros((max(1, SC_GROUPS * SC_GROUP * K),
                                        P), ml_dtypes.bfloat16)})
        colmaps.append(colmap)
    assert pos == len(vs)
    return in_maps, colmaps, nch, uvox, (NX, NY, NZ), C, B


def _aux_np():
    """Scatter idx table: token i -> part2 row i, wrapped
    16-partition-minor; pad slots -1 (trailing, ignored), unused
    partitions 0."""
    nrows = SC_GROUPS * SC_GROUP * K
    icols = -(-nrows // 16)
    aux = np.zeros((P, icols), np.int16)
    for i in range(icols * 16):
        aux[i % 16, i // 16] = i if i < nrows else -1
    return aux


def _assemble(res_core, nch, nch_cols, C):
    """[C, nch*K] fp32 partial from the two output paths of one core."""
    full = np.zeros((C, nch * K), np.float32)
    full[:, :nch_cols * K] = np.asarray(
        res_core["part"], np.float32)[:, :nch_cols * K]
    nsc = SC_GROUPS * SC_GROUP * K
    if nsc:
        blk = np.asarray(res_core["part2"], np.float32)[:nsc, :C]
        full[:, nch_cols * K:nch_cols * K + nsc] = blk.T
    return full


def _combine(parts, colmaps, uvox, dims, C, B):
    NX, NY, NZ = dims
    nu = len(uvox)
    acc = np.zeros((nu + 1, C), np.float32)
    for part, colmap in zip(parts, colmaps):
        cm = colmap.reshape(-1).copy()
        cm[cm < 0] = nu
        np.add.at(acc, cm, np.asarray(part, np.float32).T)
    total = acc[:nu].T

    out = np.zeros((B, NZ * C, NX, NY), np.float32)
    gsz = NZ * NX * NY
    b_u = uvox // gsz
    r_u = uvox % gsz
    z_u = r_u // (NX * NY)
    xy_u = r_u % (NX * NY)
    ov = out.reshape(B, NZ, C, NX * NY)
    ov[b_u, z_u, :, xy_u] = total.T
    return out


def kernel(feats, img_trans, img_scale, lidar2img):
    from concourse import bass_utils

    feats = np.ascontiguousarray(feats, dtype=np.float32)
    img_trans = np.asarray(img_trans, dtype=np.float32)
    img_scale = np.asarray(img_scale, dtype=np.float32)
    lidar2img = np.asarray(lidar2img, dtype=np.float32)
    B, N, D, H, W, C = feats.shape

    in_maps, colmaps, nch, uvox, dims, C, B = _prepare(
        feats, img_trans, img_scale, lidar2img)
    if len(uvox) == 0:
        NX, NY, NZ = dims
        return np.zeros((B, NZ * C, NX, NY), np.float32)

    nch_cols, banks, pieces = _layout(nch)
    nc = _build_bass(nch, C, nch_cols, banks, pieces)
    _split_multi_waits(nc)

    if bool(int(os.environ.get("BEV_TIMELINE", "0"))):
        from concourse.timeline_sim import TimelineSim
        t_ns = TimelineSim(nc).simulate()
        print(f"HW exec time: {t_ns:.0f} ns")

    res = bass_utils.run_bass_kernel_spmd(
        nc, in_maps, core_ids=list(range(N_CORES)))
    parts = [_assemble(r, nch, nch_cols, C) for r in res.results]
    return _combine(parts, colmaps, uvox, dims, C, B)


# revision 7
# speedup vs baseline: 1.0208x; 1.0099x over previous
"""BEV-pool (lift-splat-shoot scatter-sum) Trainium2 Bass kernel, v3.

Strategy (DMA-byte-bound problem):
  Host (index math only): voxelize every frustum point (float32 mirror of
  the reference geometry), keep the ~16% in-bounds points, sort them by
  destination voxel, and split the sorted stream into 8 per-core ranges
  balanced by *chunk count*. A chunk is <=128 consecutive points spanning
  <=K distinct voxels. Features are staged to DRAM in bf16, partition-major
  ([128, nch*C]: chunk c, point p at [p, c*C:(c+1)*C]) so every DMA moves
  >=512B contiguous runs per partition (full modeled DMA rate).

  Device (per core, shared SPMD program): stream the point superblock in
  big piece DMAs (piece 0 carries the lid/iota metadata in its leading
  columns); build all chunk one-hots with a single DVE is_equal over
  broadcast APs; one tiny bf16 matmul per chunk scatter-sums the chunk
  into its private K-wide PSUM window (start=stop=True).

  Output is two-path. Chunks in the leading 64-chunk PSUM banks use
  column-oriented windows ([C, K] per chunk), per-piece Pool copies to
  bf16 stages, and one HWDGE DMA per bank -- all hidden under the input
  stream. The tail chunks (whose data arrives last) use row-oriented
  windows ([K, C] per chunk, slots on partitions) in per-16-chunk PSUM
  tiles; their outputs go through SWDGE dma_scatter_add descriptors that
  are PREPARED early and merely TRIGGERED when each tile's copy lands,
  skipping the ~1.4us HWDGE+DGE start latency on the post-stream tail.

  Host combine: np.add.at partial columns onto their voxels (a voxel's
  points may span chunks and cores), then scatter voxel rows into the
  dense BEV grid.
"""

import os
import numpy as np
import ml_dtypes

# ---- problem constants (from the reference nn.Module) ----
IMAGE_SIZE = (256, 704)
FEATURE_SIZE = (32, 88)
XBOUND = (-54.0, 54.0, 0.3)
YBOUND = (-54.0, 54.0, 0.3)
ZBOUND = (-10.0, 10.0, 20.0)
DBOUND = (1.0, 60.0, 1.0)

N_CORES = 8
P = 128          # matmul contraction dim = points per chunk
K = 8            # psum columns (distinct voxels) per chunk window
KT = 32          # tail one-hot width (4 chunks stacked in one window)
BANK_CHUNKS = 512 // K   # chunks per col-oriented PSUM bank
SC_GROUP = 4     # tail chunks per scatter group ([32, C] psum window)
SC_GROUPS = 0    # groups routed through the single prepared scatter
SC_RING = 3      # rotating row-oriented PSUM windows
PIECE_CHUNKS = 32        # target chunks per input DMA piece


def _host_geometry(img_trans, img_scale, lidar2img, B, N, D, H, W):
    """float32 numpy mirror of the reference get_geometry + voxelize."""
    dx = np.array([XBOUND[2], YBOUND[2], ZBOUND[2]], np.float32)
    bx = np.array([XBOUND[0] + XBOUND[2] / 2.0,
                   YBOUND[0] + YBOUND[2] / 2.0,
                   ZBOUND[0] + ZBOUND[2] / 2.0], np.float32)
    nx = [int((b[1] - b[0]) / b[2]) for b in (XBOUND, YBOUND, ZBOUND)]
    NX, NY, NZ = nx

    iH, iW = IMAGE_SIZE
    fH, fW = FEATURE_SIZE
    ds = np.arange(DBOUND[0], DBOUND[1], DBOUND[2], dtype=np.float32)
    xs = np.linspace(0.0, iW - 1, fW, dtype=np.float32)
    ys = np.linspace(0.0, iH - 1, fH, dtype=np.float32)
    assert ds.shape[0] == D and fH == H and fW == W

    fr = np.stack([
        np.broadcast_to(xs[None, None, :], (D, H, W)),
        np.broadcast_to(ys[None, :, None], (D, H, W)),
        np.broadcast_to(ds[:, None, None], (D, H, W)),
    ], axis=-1).astype(np.float32)                       # [D,H,W,3]

    pts = fr[None, None] + img_trans[:, :, None, None, None, :]
    d = pts[..., 2:3]
    xy = pts[..., :2] / img_scale[:, :, None, None, None, None]
    p4 = np.concatenate([xy * d, d, np.ones_like(d)], axis=-1)
    img2lidar = np.linalg.inv(lidar2img)
    geom = np.einsum('bnij,bndhwj->bndhwi', img2lidar, p4)[..., :3]
    geom = geom.astype(np.float32)
    vox = ((geom - (bx - dx / 2.0)) / dx).astype(np.int32)  # trunc toward 0
    mask = ((vox[..., 0] >= 0) & (vox[..., 0] < NX)
            & (vox[..., 1] >= 0) & (vox[..., 1] < NY)
            & (vox[..., 2] >= 0) & (vox[..., 2] < NZ))
    flat = (vox[..., 2] * NX + vox[..., 0]) * NY + vox[..., 1]
    flat = flat + np.arange(B, dtype=np.int32)[:, None, None, None, None] \
        * (NZ * NX * NY)
    flatm = np.where(mask, flat, -1).reshape(-1)
    return flatm, (NX, NY, NZ)


def _chunk_core(run_len, run_vox, T=None):
    """Greedy chunker over a run-list. Each chunk: <=P points, <=K voxels."""
    chunks = []
    cur = []
    cap, kv = P, 0
    i, off = 0, 0
    n = len(run_len)
    while i < n:
        left = run_len[i] - off
        if left == 0:
            i += 1
            off = 0
            continue
        if cap == 0 or (kv >= K):
            chunks.append(cur)
            cur, cap, kv = [], P, 0
            if T is not None and len(chunks) >= T:
                return chunks, i, off
        take = min(left, cap)
        cur.append((run_vox[i], take))
        cap -= take
        kv += 1
        off += take
        if off == run_len[i]:
            i += 1
            off = 0
        else:
            cap = 0
    if cur:
        chunks.append(cur)
    return chunks, i, 0


def _shard(run_len, run_vox):
    """Split the global run-list into N_CORES consecutive ranges minimizing
    the max per-core chunk count."""
    total = int(np.sum(run_len))
    lo, hi = -(-total // (P * N_CORES)), 2 * -(-total // P)

    def attempt(T):
        cores = []
        rl, rv = list(run_len), list(run_vox)
        for _ in range(N_CORES):
            if not rl:
                cores.append([])
                continue
            chunks, i, off = _chunk_core(rl, rv, T)
            cores.append(chunks)
            if off > 0:
                rl, rv = [rl[i] - off] + rl[i + 1:], rv[i:]
            else:
                rl, rv = rl[i:], rv[i:]
        return cores if not rl else None

    while lo < hi:
        mid = (lo + hi) // 2
        if attempt(mid) is not None:
            hi = mid
        else:
            lo = mid + 1
    cores = attempt(lo)
    assert cores is not None
    return cores, lo


def _layout(nch):
    """Chunk-space layout. Returns (nch_cols, banks, pieces):
    banks: col-oriented [lo,hi) spans (<=BANK_CHUNKS each) covering
    [0, nch_cols); chunks [nch_cols, nch) are the SC_GROUPS scatter groups
    of SC_GROUP chunks each; pieces: input DMA spans, never crossing a
    bank or group boundary, final piece = one group (tiny tail)."""
    n_sc = SC_GROUP * SC_GROUPS
    assert nch > n_sc or not SC_GROUPS
    nch_cols = nch - n_sc
    banks = []
    lo = 0
    while lo < nch_cols:
        banks.append((lo, min(lo + BANK_CHUNKS, nch_cols)))
        lo += BANK_CHUNKS

    pieces = []
    for lo, hi in banks:
        n = hi - lo
        nsub = max(1, round(n / PIECE_CHUNKS))
        for s in range(nsub):
            a, b = lo + (n * s) // nsub, lo + (n * (s + 1)) // nsub
            if b > a:
                pieces.append((a, b))
    if not SC_GROUPS:
        return nch_cols, banks, pieces
    # early scatter groups ride as the 2nd piece; the rest arrive last
    # so only their short trigger chain tails the stream
    early = 5 * SC_GROUP
    pieces.insert(1, (nch_cols, nch_cols + early))
    g = nch_cols + early
    n = nch - g
    if n > SC_GROUP:
        pieces.append((g, g + n - SC_GROUP))
        g += n - SC_GROUP
    pieces.append((g, nch))
    return nch_cols, banks, pieces


def _build_bass(nch, C, nch_cols, banks, pieces):
    import concourse.bass as bass
    import concourse.mybir as mybir
    import concourse.tile as tile
    from concourse import library_config

    f32 = mybir.dt.float32
    bf16 = mybir.dt.bfloat16
    i16 = mybir.dt.int16
    nrows = SC_GROUPS * SC_GROUP * K  # scatter tokens / dst rows
    tpp = -(-nrows // P)             # tokens per partition
    icols = -(-nrows // 16)          # idx columns
    W0 = nch + KT + icols            # metadata columns (lid | iota | idx)
    nc = bass.Bass()
    pts = nc.dram_tensor("pts", [P, W0 + nch * C], bf16, kind="ExternalInput")
    zz = nc.dram_tensor("zz", [max(1, nrows), P], bf16,
                        kind="ExternalInput")
    part = nc.dram_tensor("part", [C, nch_cols * K], bf16,
                          kind="ExternalOutput")
    part2 = nc.dram_tensor("part2", [max(1, nrows), P], bf16,
                           kind="ExternalOutput")
    dma_sem = nc.alloc_semaphore("sc_dma_sem")
    rdy_sem = nc.alloc_semaphore("sc_rdy")
    stage2 = nc.alloc_sbuf_tensor("stage2", [P, max(1, tpp) * C], bf16)

    loc = {}
    for b, (lo, hi) in enumerate(banks):
        for c in range(lo, hi):
            loc[c] = ("col", b)
    for c in range(nch_cols, nch):
        loc[c] = ("row", (c - nch_cols) // SC_GROUP)

    with tile.TileContext(nc) as tc:
        with tc.tile_pool(name="sb", bufs=1) as con, \
             tc.tile_pool(name="ps", bufs=1, space="PSUM") as ps:
            oh = con.tile([P, nch_cols * K], bf16, tag="oh")
            ntail = nch - nch_cols
            oh2 = (con.tile([P, ntail * KT], bf16, tag="oh2")
                   if ntail else None)
            pcs = []
            for q, (a, b) in enumerate(pieces):
                w = (b - a) * C + (W0 if q == 0 else 0)
                t = con.tile([P, w], bf16, name=f"pc{q}", tag=f"pc{q}")
                pcs.append(t)
            accs = [ps.tile([P, (hi - lo) * K], f32, name=f"acc{b}",
                            tag=f"acc{b}")
                    for b, (lo, hi) in enumerate(banks)]
            stages = [con.tile([P, (hi - lo) * K], bf16, name=f"st{b}",
                               tag=f"st{b}")
                      for b, (lo, hi) in enumerate(banks)]
            ring = [ps.tile([P, C], f32, name=f"ring{r}", tag=f"ring{r}")
                    for r in range(SC_RING)]

            reloc = []
            if SC_GROUPS:
                nc.gpsimd.load_library(library_config.mlp)
                # zero the scatter-add destination (SWDGE path: keeps
                # the HWDGE free for the input pieces)
                zzi = nc.gpsimd.dma_start(out=part2[:, :], in_=zz[:])
                zz_name = zzi.ins.name

            for q, (a, b) in enumerate(pieces):
                if q == 0:
                    nc.sync.dma_start(out=pcs[q][:],
                                      in_=pts[:, 0:W0 + b * C])
                else:
                    nc.sync.dma_start(out=pcs[q][:],
                                      in_=pts[:, W0 + a * C:W0 + b * C])

            # scatter prep for the whole tail, emitted after the piece
            # DMAs (so its idx read is a RAW on piece 0, not a WAR); the
            # stage2 src read is deferred to the trigger
            if SC_GROUPS:
                st = stage2[:, :]
                in3 = bass.AP(st.tensor, st.offset,
                              [list(st.ap[0]), [C, tpp], [1, C]])
                idxs = pcs[0][:, nch + KT:nch + KT + icols].bitcast(i16)
                pr = nc.gpsimd.dma_scatter_add(
                    out_ap=part2[:, 0:C],
                    in_ap=in3,
                    idxs_ap=idxs,
                    num_idxs=nrows, num_idxs_reg=nrows, elem_size=C,
                    elem_step=P, prepare_only=True, sem=dma_sem)
                prep_name = pr.ins.name

            # one-hots in two DVE ops: oh[p, c*K+s] = (lid[p,c] == s)
            # for bank chunks; 32-wide stacked windows for tail chunks
            lid = pcs[0][:, 0:nch_cols]
            iota = pcs[0][:, nch:nch + KT]
            ap_lid = bass.AP(lid.tensor, lid.offset,
                             [list(lid.ap[0]), [1, nch_cols], [0, K]])
            ap_iota = bass.AP(iota.tensor, iota.offset,
                              [list(iota.ap[0]), [0, nch_cols], [1, K]])
            o = oh[:, :]
            ap_out = bass.AP(o.tensor, o.offset,
                             [list(o.ap[0]), [K, nch_cols], [1, K]])
            nc.vector.tensor_tensor(out=ap_out, in0=ap_lid, in1=ap_iota,
                                    op=mybir.AluOpType.is_equal)
            if ntail:
                lid2 = pcs[0][:, nch_cols:nch]
                ap_lid2 = bass.AP(lid2.tensor, lid2.offset,
                                  [list(lid2.ap[0]), [1, ntail], [0, KT]])
                ap_iota2 = bass.AP(iota.tensor, iota.offset,
                                   [list(iota.ap[0]), [0, ntail], [1, KT]])
                o2 = oh2[:, :]
                ap_out2 = bass.AP(o2.tensor, o2.offset,
                                  [list(o2.ap[0]), [KT, ntail], [1, KT]])
                nc.vector.tensor_tensor(out=ap_out2, in0=ap_lid2,
                                        in1=ap_iota2,
                                        op=mybir.AluOpType.is_equal)

            for q, (a, b) in enumerate(pieces):
                base = W0 if q == 0 else 0
                for c in range(a, b):
                    lc = c - a
                    lhs = pcs[q][:, base + lc * C:base + (lc + 1) * C]
                    kind, bi = loc[c]
                    if kind == "col":
                        blo = banks[bi][0]
                        off = (c - blo) * K
                        nc.tensor.matmul(out=accs[bi][:C, off:off + K],
                                         lhsT=lhs,
                                         rhs=oh[:, c * K:(c + 1) * K],
                                         start=True, stop=True)
                    else:
                        gc = (c - nch_cols) % SC_GROUP
                        tc_ = c - nch_cols
                        acc2 = ring[bi % SC_RING][:, :]
                        nc.tensor.matmul(
                            out=acc2[0:KT, :C],
                            lhsT=oh2[:, tc_ * KT:(tc_ + 1) * KT],
                            rhs=lhs,
                            start=(gc == 0), stop=(gc == SC_GROUP - 1))
                        if gc == SC_GROUP - 1:
                            # group complete: one copy into the token
                            # layout [128, tpp*C] (32-aligned base)
                            r0 = bi * KT
                            pr_, cb = r0 % P, r0 // P
                            od = bass.AP(stage2,
                                         pr_ * tpp * C + cb * C,
                                         [[tpp * C, KT], [1, C]])
                            if bi % 2 == 0:
                                ii = nc.scalar.activation(
                                    od, acc2[0:KT, :C],
                                    mybir.ActivationFunctionType.Copy)
                                ss = nc.scalar.sem_inc(rdy_sem, 1)
                            else:
                                ii = nc.vector.tensor_copy(
                                    out=od, in_=acc2[0:KT, :C])
                                ss = nc.vector.sem_inc(rdy_sem, 1)
                            reloc.append((ii.ins.name, ss.ins.name))
                            if bi == SC_GROUPS - 1:
                                tr = nc.gpsimd.trigger_dma(count=1)
                                trig_name = tr.ins.name
                kind, bi = loc[a]
                if kind == "col":
                    blo = banks[bi][0]
                    s0, s1 = (a - blo) * K, (b - blo) * K
                    if q % 2 == 0:
                        nc.vector.tensor_copy(out=stages[bi][:C, s0:s1],
                                              in_=accs[bi][:C, s0:s1])
                    else:
                        nc.scalar.activation(
                            stages[bi][:C, s0:s1], accs[bi][:C, s0:s1],
                            mybir.ActivationFunctionType.Copy)
                    last_bank = bi == len(banks) - 1
                    if last_bank and b == pieces[-1][0]:
                        # early columns of the final bank go out before
                        # the last piece even lands
                        nc.scalar.dma_start(
                            out=part[:, blo * K:b * K],
                            in_=stages[bi][:C, :(b - blo) * K])
                    if b == banks[bi][1]:
                        if last_bank and a > blo:
                            # only the final piece's columns tail the
                            # stream; SP's DGE delay beats Activation's
                            nc.sync.dma_start(
                                out=part[:, a * K:b * K],
                                in_=stages[bi][:C, s0:s1])
                        else:
                            lo = blo * K
                            w = (banks[bi][1] - blo) * K
                            nc.scalar.dma_start(out=part[:, lo:lo + w],
                                                in_=stages[bi][:C, :w])
            if SC_GROUPS:
                wg = nc.gpsimd.wait_ge(dma_sem, 16)
    if SC_GROUPS:
        _fix_swdge(nc, rdy_sem, dma_sem, reloc,
                   trig_name, prep_name, wg.ins.name, zz_name)
    return nc


def _fix_swdge(nc, rdy_sem, dma_sem, reloc,
               trig_name, prep_name, wg_name, zz_name):
    """Post-passes for the prepared-scatter machinery (name-based):
    0. relocate each rdy sem_inc EventSemaphore directly after its
       producing copy (the Tile scheduler hoists dep-less evsems);
    1. attach wait(rdy >= 3*SC_GROUPS) + wait(pre_sem) to the trigger
       (data deps hidden from Tile via the manual stage2 tensor);
    2. insert a pre_sem wait before the prep (idx table arrival) and
       move the wait_ge(dma_sem) right after the trigger;
    3. after each InstIncSwdgeSem (cost-model no-op), insert an explicit
       DMASW-lane bump so TimelineSim's epilogue drain isn't deadlocked
       (harmless double-bump for is_ge waits in exec mode)."""
    import concourse.mybir as mybir

    def mkwait(sem_num, name, val):
        return mybir.SyncWait(sync_type="semaphore", id=sem_num,
                              ant_name=name, wait_mode="sem-ge-imm",
                              wait_value=val, wait_reg=None)

    by_name = {}
    for bb in nc.m.functions[0].blocks:
        for inst in bb.instructions:
            by_name[inst.name] = inst

    trig = by_name[trig_name]
    ws = [mkwait(rdy_sem.num, "sc_rdy", SC_GROUPS)]
    if trig.sync_info is None:
        trig.sync_info = mybir.SyncInfo(on_wait=ws, on_update=[])
    else:
        trig.sync_info.on_wait = list(trig.sync_info.on_wait) + ws

    reloc_after = {prod: ev for prod, ev in reloc}
    ev_names = set(reloc_after.values())
    movable = {}
    for bb in nc.m.functions[0].blocks:
        for inst in bb.instructions:
            if inst.name in ev_names or inst.name == wg_name:
                movable[inst.name] = inst

    # relocate the prep cluster (IncSwdgeSem / reg moves / prep) to right
    # after the library load so its ~1us Pool engine time runs early, not
    # in the post-stream tail where the scheduler sank it
    for bb in nc.m.functions[0].blocks:
        insts = list(bb.instructions)
        try:
            pi = next(i for i, x in enumerate(insts)
                      if x.name == prep_name)
        except StopIteration:
            continue
        lo = pi
        while lo > 0 and type(insts[lo - 1]).__name__ in (
                "InstRegisterMove", "InstIncSwdgeSem"):
            lo -= 1
        cluster = insts[lo:pi + 1]
        prep = insts[pi]
        # keep Tile's engine-tick arithmetic intact: strip the tick from
        # the moved prep and fire it from a dummy at the old position
        si = prep.sync_info
        tick = [u for u in si.on_update if u.ant_name != "sc_dma_sem"]
        si.on_update = [u for u in si.on_update
                        if u.ant_name == "sc_dma_sem"]
        dummy = mybir.InstEventSemaphore(name="preptick", ins=[], outs=[])
        dummy.engine = prep.engine
        dummy.sync_info = mybir.SyncInfo(on_wait=[], on_update=tick)
        nc.inst_map[dummy.name] = dummy
        rest = insts[:lo] + [dummy] + insts[pi + 1:]
        try:
            li = next(i for i, x in enumerate(rest)
                      if x.name == zz_name)
            insts = rest[:li + 1] + cluster + rest[li + 1:]
        except StopIteration:
            insts = cluster + rest
        try:
            bb.instructions = insts
        except Exception:
            bb.instructions[:] = insts
        break

    for bb in nc.m.functions[0].blocks:
        insts = [i for i in bb.instructions if i.name not in movable]
        out = []
        for inst in insts:
            out.append(inst)
            ev = reloc_after.get(inst.name)
            if ev is not None and ev in movable:
                out.append(movable[ev])
            if inst.name == trig_name:
                out.append(movable[wg_name])
            # DMASW lane sems are a cost-model no-op at prep time (the
            # IncSwdgeSem bump never fires in TimelineSim) -- rewire any
            # wait on them to the descriptor-baked completion sem, which
            # fires in both sims
            si = inst.sync_info
            if si is not None and type(inst).__name__ == "InstDrain":
                for w in si.on_wait:
                    if w.ant_name and "DMASW" in w.ant_name:
                        w.id = dma_sem.num
                        w.ant_name = "sc_dma_sem"
                        w.wait_value = 16
        try:
            bb.instructions = out
        except Exception:
            bb.instructions[:] = out
    return nc


def _split_multi_waits(nc):
    """Walrus codegen allows a single sync-wait slot per instruction struct;
    hoist all but the last wait of any multi-wait instruction onto preceding
    single-wait EventSemaphores on the same engine queue."""
    import concourse.mybir as mybir

    k = 0
    for bb in nc.m.functions[0].blocks:
        new = []
        changed = False
        for inst in bb.instructions:
            si = inst.sync_info
            if si is not None and si.on_wait and len(si.on_wait) > 1:
                waits = list(si.on_wait)
                for w in waits[:-1]:
                    ev = mybir.InstEventSemaphore(
                        name=f"wsplit-{k}", ins=[], outs=[])
                    k += 1
                    ev.engine = inst.engine
                    ev.sync_info = mybir.SyncInfo(on_wait=[w], on_update=[])
                    nc.inst_map[ev.name] = ev
                    new.append(ev)
                si.on_wait = [waits[-1]]
                changed = True
            new.append(inst)
        if changed:
            try:
                bb.instructions = new
            except Exception:
                bb.instructions[:] = new
    return nc


def _prepare(feats, img_trans, img_scale, lidar2img):
    """Host-side indexing: geometry, sort, shard, materialize per-core
    arrays."""
    B, N, D, H, W, C = feats.shape
    npt = B * N * D * H * W

    flatm, (NX, NY, NZ) = _host_geometry(img_trans, img_scale, lidar2img,
                                         B, N, D, H, W)
    idx = np.nonzero(flatm >= 0)[0]
    keys = flatm[idx]
    order = np.argsort(keys, kind="stable")
    pidx = idx[order]
    vs = keys[order]
    uvox, run_start = np.unique(vs, return_index=True)
    run_len = np.diff(np.concatenate([run_start, [len(vs)]])).astype(int)
    run_vox = np.arange(len(uvox))

    cores, nch = _shard(list(run_len), list(run_vox))

    feats2 = feats.reshape(npt, C)
    sorted_feats = feats2[pidx].astype(ml_dtypes.bfloat16)

    iota_np = np.broadcast_to(
        np.arange(KT, dtype=np.float32)[None, :], (P, KT))
    nch_cols = nch - SC_GROUP * SC_GROUPS

    in_maps = []
    colmaps = []
    pos = 0
    for core in range(N_CORES):
        chunks = cores[core]
        arr = np.zeros((nch, P, C), ml_dtypes.bfloat16)
        lid = np.full((P, nch), -1.0, np.float32)
        colmap = np.full((nch, K), -1, np.int64)
        for c, segs in enumerate(chunks):
            n = sum(t for _, t in segs)
            arr[c, :n] = sorted_feats[pos:pos + n]
            # tail chunks stack SC_GROUP chunks into one [KT, C] psum
            # window: offset the local slot ids by gc*K
            soff = ((c - nch_cols) % SC_GROUP) * K if c >= nch_cols else 0
            o = 0
            for s, (v, t) in enumerate(segs):
                lid[o:o + t, c] = soff + s
                colmap[c, s] = v
                o += t
            pos += n
        ptsd = arr.transpose(1, 0, 2).reshape(P, nch * C)
        idx16 = np.ascontiguousarray(_aux_np()).view(ml_dtypes.bfloat16)
        meta = np.concatenate(
            [lid.astype(ml_dtypes.bfloat16),
             iota_np.astype(ml_dtypes.bfloat16), idx16], axis=1)
        pts_np = np.ascontiguousarray(np.concatenate([meta, ptsd], axis=1))
        in_maps.append({"pts": pts_np,
                        "zz": np.zeros((max(1, SC_GROUPS * SC_GROUP * K),
                                        P), ml_dtypes.bfloat16)})
        colmaps.append(colmap)
    assert pos == len(vs)
    return in_maps, colmaps, nch, uvox, (NX, NY, NZ), C, B


def _aux_np():
    """Scatter idx table: token i -> part2 row i, wrapped
    16-partition-minor; pad slots -1 (trailing, ignored), unused
    partitions 0."""
    nrows = SC_GROUPS * SC_GROUP * K
    icols = -(-nrows // 16)
    aux = np.zeros((P, icols), np.int16)
    for i in range(icols * 16):
        aux[i % 16, i // 16] = i if i < nrows else -1
    return aux


def _assemble(res_core, nch, nch_cols, C):
    """[C, nch*K] fp32 partial from the two output paths of one core."""
    full = np.zeros((C, nch * K), np.float32)
    full[:, :nch_cols * K] = np.asarray(
        res_core["part"], np.float32)[:, :nch_cols * K]
    nsc = SC_GROUPS * SC_GROUP * K
    if nsc:
        blk = np.asarray(res_core["part2"], np.float32)[:nsc, :C]
        full[:, nch_cols * K:nch_cols * K + nsc] = blk.T
    return full


def _combine(parts, colmaps, uvox, dims, C, B):
    NX, NY, NZ = dims
    nu = len(uvox)
    acc = np.zeros((nu + 1, C), np.float32)
    for part, colmap in zip(parts, colmaps):
        cm = colmap.reshape(-1).copy()
        cm[cm < 0] = nu
        np.add.at(acc, cm, np.asarray(part, np.float32).T)
    total = acc[:nu].T

    out = np.zeros((B, NZ * C, NX, NY), np.float32)
    gsz = NZ * NX * NY
    b_u = uvox // gsz
    r_u = uvox % gsz
    z_u = r_u // (NX * NY)
    xy_u = r_u % (NX * NY)
    ov = out.reshape(B, NZ, C, NX * NY)
    ov[b_u, z_u, :, xy_u] = total.T
    return out


def kernel(feats, img_trans, img_scale, lidar2img):
    from concourse import bass_utils

    feats = np.ascontiguousarray(feats, dtype=np.float32)
    img_trans = np.asarray(img_trans, dtype=np.float32)
    img_scale = np.asarray(img_scale, dtype=np.float32)
    lidar2img = np.asarray(lidar2img, dtype=np.float32)
    B, N, D, H, W, C = feats.shape

    in_maps, colmaps, nch, uvox, dims, C, B = _prepare(
        feats, img_trans, img_scale, lidar2img)
    if len(uvox) == 0:
        NX, NY, NZ = dims
        return np.zeros((B, NZ * C, NX, NY), np.float32)

    nch_cols, banks, pieces = _layout(nch)
    nc = _build_bass(nch, C, nch_cols, banks, pieces)
    _split_multi_waits(nc)

    if bool(int(os.environ.get("BEV_TIMELINE", "0"))):
        from concourse.timeline_sim import TimelineSim
        t_ns = TimelineSim(nc).simulate()
        print(f"HW exec time: {t_ns:.0f} ns")

    res = bass_utils.run_bass_kernel_spmd(
        nc, in_maps, core_ids=list(range(N_CORES)))
    parts = [_assemble(r, nch, nch_cols, C) for r in res.results]
    return _combine(parts, colmaps, uvox, dims, C, B)


# revision 8
# speedup vs baseline: 1.0410x; 1.0198x over previous
"""BEV-pool (lift-splat-shoot scatter-sum) Trainium2 Bass kernel, v3.

Strategy (DMA-byte-bound problem):
  Host (index math only): voxelize every frustum point (float32 mirror of
  the reference geometry), keep the ~16% in-bounds points, sort them by
  destination voxel, and split the sorted stream into 8 per-core ranges
  balanced by *chunk count*. A chunk is <=128 consecutive points spanning
  <=K distinct voxels. Features are staged to DRAM in bf16, partition-major
  ([128, nch*C]: chunk c, point p at [p, c*C:(c+1)*C]) so every DMA moves
  >=512B contiguous runs per partition (full modeled DMA rate).

  Device (per core, shared SPMD program): stream the point superblock in
  big piece DMAs (piece 0 carries the lid/iota metadata in its leading
  columns); build all chunk one-hots with a single DVE is_equal over
  broadcast APs; one tiny bf16 matmul per chunk scatter-sums the chunk
  into its private K-wide PSUM window (start=stop=True).

  Output is two-path. Chunks in the leading 64-chunk PSUM banks use
  column-oriented windows ([C, K] per chunk), per-piece Pool copies to
  bf16 stages, and one HWDGE DMA per bank -- all hidden under the input
  stream. The tail chunks (whose data arrives last) use row-oriented
  windows ([K, C] per chunk, slots on partitions) in per-16-chunk PSUM
  tiles; their outputs go through SWDGE dma_scatter_add descriptors that
  are PREPARED early and merely TRIGGERED when each tile's copy lands,
  skipping the ~1.4us HWDGE+DGE start latency on the post-stream tail.

  Host combine: np.add.at partial columns onto their voxels (a voxel's
  points may span chunks and cores), then scatter voxel rows into the
  dense BEV grid.
"""

import os
import numpy as np
import ml_dtypes

# ---- problem constants (from the reference nn.Module) ----
IMAGE_SIZE = (256, 704)
FEATURE_SIZE = (32, 88)
XBOUND = (-54.0, 54.0, 0.3)
YBOUND = (-54.0, 54.0, 0.3)
ZBOUND = (-10.0, 10.0, 20.0)
DBOUND = (1.0, 60.0, 1.0)

N_CORES = 8
P = 128          # matmul contraction dim = points per chunk
K = 8            # psum columns (distinct voxels) per chunk window
KT = 32          # tail one-hot width (4 chunks stacked in one window)
BANK_CHUNKS = 512 // K   # chunks per col-oriented PSUM bank
SC_GROUP = 4     # tail chunks per scatter group ([32, C] psum window)
SC_GROUPS = 0    # groups routed through the single prepared scatter
SC_RING = 3      # rotating row-oriented PSUM windows
PIECE_CHUNKS = 32        # target chunks per input DMA piece


def _host_geometry(img_trans, img_scale, lidar2img, B, N, D, H, W):
    """float32 numpy mirror of the reference get_geometry + voxelize."""
    dx = np.array([XBOUND[2], YBOUND[2], ZBOUND[2]], np.float32)
    bx = np.array([XBOUND[0] + XBOUND[2] / 2.0,
                   YBOUND[0] + YBOUND[2] / 2.0,
                   ZBOUND[0] + ZBOUND[2] / 2.0], np.float32)
    nx = [int((b[1] - b[0]) / b[2]) for b in (XBOUND, YBOUND, ZBOUND)]
    NX, NY, NZ = nx

    iH, iW = IMAGE_SIZE
    fH, fW = FEATURE_SIZE
    ds = np.arange(DBOUND[0], DBOUND[1], DBOUND[2], dtype=np.float32)
    xs = np.linspace(0.0, iW - 1, fW, dtype=np.float32)
    ys = np.linspace(0.0, iH - 1, fH, dtype=np.float32)
    assert ds.shape[0] == D and fH == H and fW == W

    fr = np.stack([
        np.broadcast_to(xs[None, None, :], (D, H, W)),
        np.broadcast_to(ys[None, :, None], (D, H, W)),
        np.broadcast_to(ds[:, None, None], (D, H, W)),
    ], axis=-1).astype(np.float32)                       # [D,H,W,3]

    pts = fr[None, None] + img_trans[:, :, None, None, None, :]
    d = pts[..., 2:3]
    xy = pts[..., :2] / img_scale[:, :, None, None, None, None]
    p4 = np.concatenate([xy * d, d, np.ones_like(d)], axis=-1)
    img2lidar = np.linalg.inv(lidar2img)
    geom = np.einsum('bnij,bndhwj->bndhwi', img2lidar, p4)[..., :3]
    geom = geom.astype(np.float32)
    vox = ((geom - (bx - dx / 2.0)) / dx).astype(np.int32)  # trunc toward 0
    mask = ((vox[..., 0] >= 0) & (vox[..., 0] < NX)
            & (vox[..., 1] >= 0) & (vox[..., 1] < NY)
            & (vox[..., 2] >= 0) & (vox[..., 2] < NZ))
    flat = (vox[..., 2] * NX + vox[..., 0]) * NY + vox[..., 1]
    flat = flat + np.arange(B, dtype=np.int32)[:, None, None, None, None] \
        * (NZ * NX * NY)
    flatm = np.where(mask, flat, -1).reshape(-1)
    return flatm, (NX, NY, NZ)


def _chunk_core(run_len, run_vox, T=None):
    """Greedy chunker over a run-list. Each chunk: <=P points, <=K voxels."""
    chunks = []
    cur = []
    cap, kv = P, 0
    i, off = 0, 0
    n = len(run_len)
    while i < n:
        left = run_len[i] - off
        if left == 0:
            i += 1
            off = 0
            continue
        if cap == 0 or (kv >= K):
            chunks.append(cur)
            cur, cap, kv = [], P, 0
            if T is not None and len(chunks) >= T:
                return chunks, i, off
        take = min(left, cap)
        cur.append((run_vox[i], take))
        cap -= take
        kv += 1
        off += take
        if off == run_len[i]:
            i += 1
            off = 0
        else:
            cap = 0
    if cur:
        chunks.append(cur)
    return chunks, i, 0


def _shard(run_len, run_vox):
    """Split the global run-list into N_CORES consecutive ranges minimizing
    the max per-core chunk count."""
    total = int(np.sum(run_len))
    lo, hi = -(-total // (P * N_CORES)), 2 * -(-total // P)

    def attempt(T):
        cores = []
        rl, rv = list(run_len), list(run_vox)
        for _ in range(N_CORES):
            if not rl:
                cores.append([])
                continue
            chunks, i, off = _chunk_core(rl, rv, T)
            cores.append(chunks)
            if off > 0:
                rl, rv = [rl[i] - off] + rl[i + 1:], rv[i:]
            else:
                rl, rv = rl[i:], rv[i:]
        return cores if not rl else None

    while lo < hi:
        mid = (lo + hi) // 2
        if attempt(mid) is not None:
            hi = mid
        else:
            lo = mid + 1
    cores = attempt(lo)
    assert cores is not None
    return cores, lo


def _layout(nch):
    """Chunk-space layout. Returns (nch_cols, banks, pieces):
    banks: col-oriented [lo,hi) spans (<=BANK_CHUNKS each) covering
    [0, nch_cols); chunks [nch_cols, nch) are the SC_GROUPS scatter groups
    of SC_GROUP chunks each; pieces: input DMA spans, never crossing a
    bank or group boundary, final piece = one group (tiny tail)."""
    n_sc = SC_GROUP * SC_GROUPS
    assert nch > n_sc or not SC_GROUPS
    nch_cols = nch - n_sc
    banks = []
    lo = 0
    while lo < nch_cols:
        banks.append((lo, min(lo + BANK_CHUNKS, nch_cols)))
        lo += BANK_CHUNKS

    pieces = []
    for lo, hi in banks:
        n = hi - lo
        nsub = max(1, round(n / PIECE_CHUNKS))
        for s in range(nsub):
            a, b = lo + (n * s) // nsub, lo + (n * (s + 1)) // nsub
            if b > a:
                pieces.append((a, b))
    if not SC_GROUPS:
        return nch_cols, banks, pieces
    # early scatter groups ride as the 2nd piece; the rest arrive last
    # so only their short trigger chain tails the stream
    early = 5 * SC_GROUP
    pieces.insert(1, (nch_cols, nch_cols + early))
    g = nch_cols + early
    n = nch - g
    if n > SC_GROUP:
        pieces.append((g, g + n - SC_GROUP))
        g += n - SC_GROUP
    pieces.append((g, nch))
    return nch_cols, banks, pieces


def _build_bass(nch, C, nch_cols, banks, pieces):
    import concourse.bass as bass
    import concourse.mybir as mybir
    import concourse.tile as tile
    from concourse import library_config

    f32 = mybir.dt.float32
    bf16 = mybir.dt.bfloat16
    i16 = mybir.dt.int16
    nrows = SC_GROUPS * SC_GROUP * K  # scatter tokens / dst rows
    tpp = -(-nrows // P)             # tokens per partition
    icols = -(-nrows // 16)          # idx columns
    W0 = nch + KT + icols            # metadata columns (lid | iota | idx)
    nc = bass.Bass()
    pts = nc.dram_tensor("pts", [P, W0 + nch * C], bf16, kind="ExternalInput")
    zz = nc.dram_tensor("zz", [max(1, nrows), P], bf16,
                        kind="ExternalInput")
    part = nc.dram_tensor("part", [C, nch_cols * K], bf16,
                          kind="ExternalOutput")
    part2 = nc.dram_tensor("part2", [max(1, nrows), P], bf16,
                           kind="ExternalOutput")
    dma_sem = nc.alloc_semaphore("sc_dma_sem")
    rdy_sem = nc.alloc_semaphore("sc_rdy")
    stage2 = nc.alloc_sbuf_tensor("stage2", [P, max(1, tpp) * C], bf16)

    loc = {}
    for b, (lo, hi) in enumerate(banks):
        for c in range(lo, hi):
            loc[c] = ("col", b)
    for c in range(nch_cols, nch):
        loc[c] = ("row", (c - nch_cols) // SC_GROUP)

    with tile.TileContext(nc) as tc:
        with tc.tile_pool(name="sb", bufs=1) as con, \
             tc.tile_pool(name="ps", bufs=1, space="PSUM") as ps:
            oh = con.tile([P, nch_cols * K], bf16, tag="oh")
            ntail = nch - nch_cols
            oh2 = (con.tile([P, ntail * KT], bf16, tag="oh2")
                   if ntail else None)
            pcs = []
            for q, (a, b) in enumerate(pieces):
                w = (b - a) * C + (W0 if q == 0 else 0)
                t = con.tile([P, w], bf16, name=f"pc{q}", tag=f"pc{q}")
                pcs.append(t)
            accs = [ps.tile([P, (hi - lo) * K], f32, name=f"acc{b}",
                            tag=f"acc{b}")
                    for b, (lo, hi) in enumerate(banks)]
            stages = [con.tile([P, (hi - lo) * K], bf16, name=f"st{b}",
                               tag=f"st{b}")
                      for b, (lo, hi) in enumerate(banks)]
            ring = [ps.tile([P, C], f32, name=f"ring{r}", tag=f"ring{r}")
                    for r in range(SC_RING)]

            reloc = []
            if SC_GROUPS:
                nc.gpsimd.load_library(library_config.mlp)
                # zero the scatter-add destination (SWDGE path: keeps
                # the HWDGE free for the input pieces)
                zzi = nc.gpsimd.dma_start(out=part2[:, :], in_=zz[:])
                zz_name = zzi.ins.name

            for q, (a, b) in enumerate(pieces):
                if q == 0:
                    nc.sync.dma_start(out=pcs[q][:],
                                      in_=pts[:, 0:W0 + b * C])
                else:
                    nc.sync.dma_start(out=pcs[q][:],
                                      in_=pts[:, W0 + a * C:W0 + b * C])

            # scatter prep for the whole tail, emitted after the piece
            # DMAs (so its idx read is a RAW on piece 0, not a WAR); the
            # stage2 src read is deferred to the trigger
            if SC_GROUPS:
                st = stage2[:, :]
                in3 = bass.AP(st.tensor, st.offset,
                              [list(st.ap[0]), [C, tpp], [1, C]])
                idxs = pcs[0][:, nch + KT:nch + KT + icols].bitcast(i16)
                pr = nc.gpsimd.dma_scatter_add(
                    out_ap=part2[:, 0:C],
                    in_ap=in3,
                    idxs_ap=idxs,
                    num_idxs=nrows, num_idxs_reg=nrows, elem_size=C,
                    elem_step=P, prepare_only=True, sem=dma_sem)
                prep_name = pr.ins.name

            # one-hots in two DVE ops: oh[p, c*K+s] = (lid[p,c] == s)
            # for bank chunks; 32-wide stacked windows for tail chunks
            lid = pcs[0][:, 0:nch_cols]
            iota = pcs[0][:, nch:nch + KT]
            ap_lid = bass.AP(lid.tensor, lid.offset,
                             [list(lid.ap[0]), [1, nch_cols], [0, K]])
            ap_iota = bass.AP(iota.tensor, iota.offset,
                              [list(iota.ap[0]), [0, nch_cols], [1, K]])
            o = oh[:, :]
            ap_out = bass.AP(o.tensor, o.offset,
                             [list(o.ap[0]), [K, nch_cols], [1, K]])
            nc.vector.tensor_tensor(out=ap_out, in0=ap_lid, in1=ap_iota,
                                    op=mybir.AluOpType.is_equal)
            if ntail:
                lid2 = pcs[0][:, nch_cols:nch]
                ap_lid2 = bass.AP(lid2.tensor, lid2.offset,
                                  [list(lid2.ap[0]), [1, ntail], [0, KT]])
                ap_iota2 = bass.AP(iota.tensor, iota.offset,
                                   [list(iota.ap[0]), [0, ntail], [1, KT]])
                o2 = oh2[:, :]
                ap_out2 = bass.AP(o2.tensor, o2.offset,
                                  [list(o2.ap[0]), [KT, ntail], [1, KT]])
                nc.vector.tensor_tensor(out=ap_out2, in0=ap_lid2,
                                        in1=ap_iota2,
                                        op=mybir.AluOpType.is_equal)

            for q, (a, b) in enumerate(pieces):
                base = W0 if q == 0 else 0
                for c in range(a, b):
                    lc = c - a
                    lhs = pcs[q][:, base + lc * C:base + (lc + 1) * C]
                    kind, bi = loc[c]
                    if kind == "col":
                        blo = banks[bi][0]
                        off = (c - blo) * K
                        nc.tensor.matmul(out=accs[bi][:C, off:off + K],
                                         lhsT=lhs,
                                         rhs=oh[:, c * K:(c + 1) * K],
                                         start=True, stop=True)
                    else:
                        gc = (c - nch_cols) % SC_GROUP
                        tc_ = c - nch_cols
                        acc2 = ring[bi % SC_RING][:, :]
                        nc.tensor.matmul(
                            out=acc2[0:KT, :C],
                            lhsT=oh2[:, tc_ * KT:(tc_ + 1) * KT],
                            rhs=lhs,
                            start=(gc == 0), stop=(gc == SC_GROUP - 1))
                        if gc == SC_GROUP - 1:
                            # group complete: one copy into the token
                            # layout [128, tpp*C] (32-aligned base)
                            r0 = bi * KT
                            pr_, cb = r0 % P, r0 // P
                            od = bass.AP(stage2,
                                         pr_ * tpp * C + cb * C,
                                         [[tpp * C, KT], [1, C]])
                            if bi % 2 == 0:
                                ii = nc.scalar.activation(
                                    od, acc2[0:KT, :C],
                                    mybir.ActivationFunctionType.Copy)
                                ss = nc.scalar.sem_inc(rdy_sem, 1)
                            else:
                                ii = nc.vector.tensor_copy(
                                    out=od, in_=acc2[0:KT, :C])
                                ss = nc.vector.sem_inc(rdy_sem, 1)
                            reloc.append((ii.ins.name, ss.ins.name))
                            if bi == SC_GROUPS - 1:
                                tr = nc.gpsimd.trigger_dma(count=1)
                                trig_name = tr.ins.name
                kind, bi = loc[a]
                if kind == "col":
                    blo = banks[bi][0]
                    s0, s1 = (a - blo) * K, (b - blo) * K
                    if q % 2 == 0:
                        nc.vector.tensor_copy(out=stages[bi][:C, s0:s1],
                                              in_=accs[bi][:C, s0:s1])
                    else:
                        nc.scalar.activation(
                            stages[bi][:C, s0:s1], accs[bi][:C, s0:s1],
                            mybir.ActivationFunctionType.Copy)
                    last_bank = bi == len(banks) - 1
                    if last_bank and b == pieces[-1][0]:
                        # early columns of the final bank go out before
                        # the last piece even lands
                        nc.scalar.dma_start(
                            out=part[:, blo * K:b * K],
                            in_=stages[bi][:C, :(b - blo) * K])
                    if b == banks[bi][1]:
                        if last_bank and a > blo:
                            # only the final piece's columns tail the
                            # stream; SP's DGE delay beats Activation's
                            nc.sync.dma_start(
                                out=part[:, a * K:b * K],
                                in_=stages[bi][:C, s0:s1])
                        else:
                            lo = blo * K
                            w = (banks[bi][1] - blo) * K
                            nc.scalar.dma_start(out=part[:, lo:lo + w],
                                                in_=stages[bi][:C, :w])
            if SC_GROUPS:
                wg = nc.gpsimd.wait_ge(dma_sem, 16)
    if SC_GROUPS:
        _fix_swdge(nc, rdy_sem, dma_sem, reloc,
                   trig_name, prep_name, wg.ins.name, zz_name)
    return nc


def _fix_swdge(nc, rdy_sem, dma_sem, reloc,
               trig_name, prep_name, wg_name, zz_name):
    """Post-passes for the prepared-scatter machinery (name-based):
    0. relocate each rdy sem_inc EventSemaphore directly after its
       producing copy (the Tile scheduler hoists dep-less evsems);
    1. attach wait(rdy >= 3*SC_GROUPS) + wait(pre_sem) to the trigger
       (data deps hidden from Tile via the manual stage2 tensor);
    2. insert a pre_sem wait before the prep (idx table arrival) and
       move the wait_ge(dma_sem) right after the trigger;
    3. after each InstIncSwdgeSem (cost-model no-op), insert an explicit
       DMASW-lane bump so TimelineSim's epilogue drain isn't deadlocked
       (harmless double-bump for is_ge waits in exec mode)."""
    import concourse.mybir as mybir

    def mkwait(sem_num, name, val):
        return mybir.SyncWait(sync_type="semaphore", id=sem_num,
                              ant_name=name, wait_mode="sem-ge-imm",
                              wait_value=val, wait_reg=None)

    by_name = {}
    for bb in nc.m.functions[0].blocks:
        for inst in bb.instructions:
            by_name[inst.name] = inst

    trig = by_name[trig_name]
    ws = [mkwait(rdy_sem.num, "sc_rdy", SC_GROUPS)]
    if trig.sync_info is None:
        trig.sync_info = mybir.SyncInfo(on_wait=ws, on_update=[])
    else:
        trig.sync_info.on_wait = list(trig.sync_info.on_wait) + ws

    reloc_after = {prod: ev for prod, ev in reloc}
    ev_names = set(reloc_after.values())
    movable = {}
    for bb in nc.m.functions[0].blocks:
        for inst in bb.instructions:
            if inst.name in ev_names or inst.name == wg_name:
                movable[inst.name] = inst

    # relocate the prep cluster (IncSwdgeSem / reg moves / prep) to right
    # after the library load so its ~1us Pool engine time runs early, not
    # in the post-stream tail where the scheduler sank it
    for bb in nc.m.functions[0].blocks:
        insts = list(bb.instructions)
        try:
            pi = next(i for i, x in enumerate(insts)
                      if x.name == prep_name)
        except StopIteration:
            continue
        lo = pi
        while lo > 0 and type(insts[lo - 1]).__name__ in (
                "InstRegisterMove", "InstIncSwdgeSem"):
            lo -= 1
        cluster = insts[lo:pi + 1]
        prep = insts[pi]
        # keep Tile's engine-tick arithmetic intact: strip the tick from
        # the moved prep and fire it from a dummy at the old position
        si = prep.sync_info
        tick = [u for u in si.on_update if u.ant_name != "sc_dma_sem"]
        si.on_update = [u for u in si.on_update
                        if u.ant_name == "sc_dma_sem"]
        dummy = mybir.InstEventSemaphore(name="preptick", ins=[], outs=[])
        dummy.engine = prep.engine
        dummy.sync_info = mybir.SyncInfo(on_wait=[], on_update=tick)
        nc.inst_map[dummy.name] = dummy
        rest = insts[:lo] + [dummy] + insts[pi + 1:]
        try:
            li = next(i for i, x in enumerate(rest)
                      if x.name == zz_name)
            insts = rest[:li + 1] + cluster + rest[li + 1:]
        except StopIteration:
            insts = cluster + rest
        try:
            bb.instructions = insts
        except Exception:
            bb.instructions[:] = insts
        break

    for bb in nc.m.functions[0].blocks:
        insts = [i for i in bb.instructions if i.name not in movable]
        out = []
        for inst in insts:
            out.append(inst)
            ev = reloc_after.get(inst.name)
            if ev is not None and ev in movable:
                out.append(movable[ev])
            if inst.name == trig_name:
                out.append(movable[wg_name])
            # DMASW lane sems are a cost-model no-op at prep time (the
            # IncSwdgeSem bump never fires in TimelineSim) -- rewire any
            # wait on them to the descriptor-baked completion sem, which
            # fires in both sims
            si = inst.sync_info
            if si is not None and type(inst).__name__ == "InstDrain":
                for w in si.on_wait:
                    if w.ant_name and "DMASW" in w.ant_name:
                        w.id = dma_sem.num
                        w.ant_name = "sc_dma_sem"
                        w.wait_value = 16
        try:
            bb.instructions = out
        except Exception:
            bb.instructions[:] = out
    return nc


def _strip_dead_consts(nc):
    """The Bass prolog memsets four const tensors this kernel never
    references; they gate the all-engine start barrier by ~0.4us."""
    for bb in nc.m.functions[0].blocks:
        keep = [i for i in bb.instructions
                if not (type(i).__name__ == "InstMemset"
                        and "const-" in str(i))]
        if len(keep) != len(bb.instructions):
            try:
                bb.instructions = keep
            except Exception:
                bb.instructions[:] = keep
    return nc


def _split_multi_waits(nc):
    """Walrus codegen allows a single sync-wait slot per instruction struct;
    hoist all but the last wait of any multi-wait instruction onto preceding
    single-wait EventSemaphores on the same engine queue."""
    import concourse.mybir as mybir

    k = 0
    for bb in nc.m.functions[0].blocks:
        new = []
        changed = False
        for inst in bb.instructions:
            si = inst.sync_info
            if si is not None and si.on_wait and len(si.on_wait) > 1:
                waits = list(si.on_wait)
                for w in waits[:-1]:
                    ev = mybir.InstEventSemaphore(
                        name=f"wsplit-{k}", ins=[], outs=[])
                    k += 1
                    ev.engine = inst.engine
                    ev.sync_info = mybir.SyncInfo(on_wait=[w], on_update=[])
                    nc.inst_map[ev.name] = ev
                    new.append(ev)
                si.on_wait = [waits[-1]]
                changed = True
            new.append(inst)
        if changed:
            try:
                bb.instructions = new
            except Exception:
                bb.instructions[:] = new
    return nc


def _prepare(feats, img_trans, img_scale, lidar2img):
    """Host-side indexing: geometry, sort, shard, materialize per-core
    arrays."""
    B, N, D, H, W, C = feats.shape
    npt = B * N * D * H * W

    flatm, (NX, NY, NZ) = _host_geometry(img_trans, img_scale, lidar2img,
                                         B, N, D, H, W)
    idx = np.nonzero(flatm >= 0)[0]
    keys = flatm[idx]
    order = np.argsort(keys, kind="stable")
    pidx = idx[order]
    vs = keys[order]
    uvox, run_start = np.unique(vs, return_index=True)
    run_len = np.diff(np.concatenate([run_start, [len(vs)]])).astype(int)
    run_vox = np.arange(len(uvox))

    cores, nch = _shard(list(run_len), list(run_vox))

    feats2 = feats.reshape(npt, C)
    sorted_feats = feats2[pidx].astype(ml_dtypes.bfloat16)

    iota_np = np.broadcast_to(
        np.arange(KT, dtype=np.float32)[None, :], (P, KT))
    nch_cols = nch - SC_GROUP * SC_GROUPS

    in_maps = []
    colmaps = []
    pos = 0
    for core in range(N_CORES):
        chunks = cores[core]
        arr = np.zeros((nch, P, C), ml_dtypes.bfloat16)
        lid = np.full((P, nch), -1.0, np.float32)
        colmap = np.full((nch, K), -1, np.int64)
        for c, segs in enumerate(chunks):
            n = sum(t for _, t in segs)
            arr[c, :n] = sorted_feats[pos:pos + n]
            # tail chunks stack SC_GROUP chunks into one [KT, C] psum
            # window: offset the local slot ids by gc*K
            soff = ((c - nch_cols) % SC_GROUP) * K if c >= nch_cols else 0
            o = 0
            for s, (v, t) in enumerate(segs):
                lid[o:o + t, c] = soff + s
                colmap[c, s] = v
                o += t
            pos += n
        ptsd = arr.transpose(1, 0, 2).reshape(P, nch * C)
        idx16 = np.ascontiguousarray(_aux_np()).view(ml_dtypes.bfloat16)
        meta = np.concatenate(
            [lid.astype(ml_dtypes.bfloat16),
             iota_np.astype(ml_dtypes.bfloat16), idx16], axis=1)
        pts_np = np.ascontiguousarray(np.concatenate([meta, ptsd], axis=1))
        in_maps.append({"pts": pts_np,
                        "zz": np.zeros((max(1, SC_GROUPS * SC_GROUP * K),
                                        P), ml_dtypes.bfloat16)})
        colmaps.append(colmap)
    assert pos == len(vs)
    return in_maps, colmaps, nch, uvox, (NX, NY, NZ), C, B


def _aux_np():
    """Scatter idx table: token i -> part2 row i, wrapped
    16-partition-minor; pad slots -1 (trailing, ignored), unused
    partitions 0."""
    nrows = SC_GROUPS * SC_GROUP * K
    icols = -(-nrows // 16)
    aux = np.zeros((P, icols), np.int16)
    for i in range(icols * 16):
        aux[i % 16, i // 16] = i if i < nrows else -1
    return aux


def _assemble(res_core, nch, nch_cols, C):
    """[C, nch*K] fp32 partial from the two output paths of one core."""
    full = np.zeros((C, nch * K), np.float32)
    full[:, :nch_cols * K] = np.asarray(
        res_core["part"], np.float32)[:, :nch_cols * K]
    nsc = SC_GROUPS * SC_GROUP * K
    if nsc:
        blk = np.asarray(res_core["part2"], np.float32)[:nsc, :C]
        full[:, nch_cols * K:nch_cols * K + nsc] = blk.T
    return full


def _combine(parts, colmaps, uvox, dims, C, B):
    NX, NY, NZ = dims
    nu = len(uvox)
    acc = np.zeros((nu + 1, C), np.float32)
    for part, colmap in zip(parts, colmaps):
        cm = colmap.reshape(-1).copy()
        cm[cm < 0] = nu
        np.add.at(acc, cm, np.asarray(part, np.float32).T)
    total = acc[:nu].T

    out = np.zeros((B, NZ * C, NX, NY), np.float32)
    gsz = NZ * NX * NY
    b_u = uvox // gsz
    r_u = uvox % gsz
    z_u = r_u // (NX * NY)
    xy_u = r_u % (NX * NY)
    ov = out.reshape(B, NZ, C, NX * NY)
    ov[b_u, z_u, :, xy_u] = total.T
    return out


def kernel(feats, img_trans, img_scale, lidar2img):
    from concourse import bass_utils

    feats = np.ascontiguousarray(feats, dtype=np.float32)
    img_trans = np.asarray(img_trans, dtype=np.float32)
    img_scale = np.asarray(img_scale, dtype=np.float32)
    lidar2img = np.asarray(lidar2img, dtype=np.float32)
    B, N, D, H, W, C = feats.shape

    in_maps, colmaps, nch, uvox, dims, C, B = _prepare(
        feats, img_trans, img_scale, lidar2img)
    if len(uvox) == 0:
        NX, NY, NZ = dims
        return np.zeros((B, NZ * C, NX, NY), np.float32)

    nch_cols, banks, pieces = _layout(nch)
    nc = _build_bass(nch, C, nch_cols, banks, pieces)
    _strip_dead_consts(nc)
    _split_multi_waits(nc)

    if bool(int(os.environ.get("BEV_TIMELINE", "0"))):
        from concourse.timeline_sim import TimelineSim
        t_ns = TimelineSim(nc).simulate()
        print(f"HW exec time: {t_ns:.0f} ns")

    res = bass_utils.run_bass_kernel_spmd(
        nc, in_maps, core_ids=list(range(N_CORES)))
    parts = [_assemble(r, nch, nch_cols, C) for r in res.results]
    return _combine(parts, colmaps, uvox, dims, C, B)
